# revision 12
# baseline (speedup 1.0000x reference)
"""Causal self-attention kernel for 8 Trainium2 NeuronCores.

Sharding: core c -> (batch b = c//2, head-group g = c%2). Each core computes
the attention output contribution of 8 heads for one batch element:
    P_c = (sum_{h in group} softmax(Q_h K_h^T / 8 + causal) V_h) @ WO
Host epilogue: out[b] = P_{2b} + P_{2b+1} + (sum_h bV_h) @ WO + 16*bO
(the V-bias commutes through softmax normalization: softmax rows sum to 1).

Precision/speed split (PE row counts at ~1.5 G rows/s are the limiter):
  - Q/K/V projections run as fp8-E4M3 DoubleRow matmuls (K=256 per pass:
    d-tile pairs via a [128, 2, N] AP over the existing xt layout), halving
    projection PE time. Chunk 0's V is also computed in fp16 (kept in vt16)
    because output rows with tiny softmax support (q < 512) see V error
    directly; all other chunks' softmax averages 512+ values so fp8 noise
    washes out.
  - Scores stay fp16 (K=64 contraction gets no DoubleRow benefit); two heads
    per PE pass via tile_position row-tiling. Q/K fp8 projection error only
    perturbs softmax weights (benign at any support size).
  - A@V for q-chunks >= 1 runs fp8 DoubleRow over k-tile PAIRS: ET pair
    tiles [128, 2, 2head*512] written by exp directly in fp8, V pair tiles
    [128, 2, 8head*65] with a ones column accumulating the softmax
    denominator in row 64. Chunk 0 uses the fp16 per-k-tile path.
  - Diagonal k-tiles shrink the ST matmul, exp, and ZT to the unmasked
    q-window; fully masked regions are never computed (no memsets except the
    odd-slot delta of diagonal fp8 pairs). The in-block triangle is masked
    by gpsimd affine_select on ET after exp.
  - Per-hp tail: 1/l via DVE reciprocal_approx_fast (no ScalarE Ln/Exp table
    swaps), broadcast by a DRAM-bounce DMA, normalize straight out of zt
    PSUM into the zsum accumulator, so almost nothing serializes at the end.
"""
import numpy as np

B, S, D, H, DH = 4, 2048, 1024, 16, 64
HPC = 8            # heads per core
GD = HPC * DH      # 512 = group width
NCORES = 8
NQ = S // 512      # 4 q/s chunks of 512
NKT = S // 128     # 16 k-tiles
NDT = D // 128     # 8 d-tiles

_prog = {}


def ap3(tile_t, offset, d1, n1, d2, n2):
    """AP view [128p, n1, n2] over a tile's free dim: col = offset + i*d1 + j*d2."""
    import concourse.bass as bass
    ap = tile_t[:]
    return bass.AP(ap.tensor, ap.offset + offset,
                   [ap.ap[0], [d1, n1], [d2, n2]])


def bass_ap_3d(tile_t, offset, stride, n, inner):
    return ap3(tile_t, offset, stride, n, 1, inner)


def _bcast_ap(tile_t, row, col, nparts, width):
    """Partition-step-0 AP reading (row, col:col+width) replicated nparts times."""
    import concourse.bass as bass
    ap = tile_t[:]
    pstep = ap.ap[0][0]
    return bass.AP(ap.tensor, ap.offset + row * pstep + col,
                   [[0, nparts], [1, width]])


def _build():
    import concourse.bacc as bacc
    import concourse.tile as tile
    from concourse import mybir
    import concourse.bass as bass

    f32 = mybir.dt.float32
    f16 = mybir.dt.float16
    f8 = mybir.dt.float8e4
    AF = mybir.ActivationFunctionType
    ALU = mybir.AluOpType
    DR = mybir.MatmulPerfMode.DoubleRow

    nc = bacc.Bacc(None, target_bir_lowering=False, debug=False)
    x = nc.dram_tensor("x", [S, D], f16, kind="ExternalInput")
    # wq8/wk8/wv8: [4*128, 2*512]: row = g*128 + p, col = par*512 + m,
    # value = W[g*256 + par*128 + p, m]  (d-pair DoubleRow layout)
    wq8 = nc.dram_tensor("wq8", [512, 1024], f8, kind="ExternalInput")
    wk8 = nc.dram_tensor("wk8", [512, 1024], f8, kind="ExternalInput")
    wv8 = nc.dram_tensor("wv8", [512, 1024], f8, kind="ExternalInput")
    wv16 = nc.dram_tensor("wv16", [D, GD], f16, kind="ExternalInput")
    bq = nc.dram_tensor("bq", [1, GD], f16, kind="ExternalInput")
    bk = nc.dram_tensor("bk", [1, GD], f16, kind="ExternalInput")
    wo = nc.dram_tensor("wo", [DH, D], f16, kind="ExternalInput")
    out = nc.dram_tensor("out", [S, D], f32, kind="ExternalOutput")

    with tile.TileContext(nc) as tc:
        with tc.tile_pool(name="const", bufs=1) as constp, \
             tc.tile_pool(name="big", bufs=1) as bigp:
            idt16 = constp.tile([128, 128], f16, tag="idt16")
            from concourse.masks import make_identity
            make_identity(nc, idt16[:])
            bq_t = constp.tile([128, 4], f32, tag="bq_t")
            bk_t = constp.tile([128, 4], f32, tag="bk_t")
            nc.gpsimd.dma_start(bq_t[:], bass.AP(bq, 0, [[1, 128], [128, 4]]))
            nc.gpsimd.dma_start(bk_t[:], bass.AP(bk, 0, [[1, 128], [128, 4]]))
            wo_sb = constp.tile([128, D], f16, tag="wo_sb")
            nc.gpsimd.dma_start(wo_sb[0:DH, :], wo[:])
            nc.gpsimd.dma_start(wo_sb[DH:2 * DH, :], wo[:])

            # persistent per-core tensors
            xt8_all = bigp.tile([128, NDT * S], f8, tag="xt8")    # d-tile j at j*S
            xt16_c0 = bigp.tile([128, NDT * 512], f16, tag="xt16")  # d-tile j at j*512
            qt_all = bigp.tile([128, 4 * S], f16, tag="qt")       # m-tile m at m*S
            kt_all = bigp.tile([128, 4 * S], f16, tag="kt")
            # vt8: k-tile pair p: [par(2) x head(8) x 65] at col par*520+h*65+c
            vt8 = [bigp.tile([128, 1280], f8, tag=f"vt8_{p}", name=f"vt8_{p}")
                   for p in range(NKT // 2)]
            vt16 = bigp.tile([128, 4 * 520], f16, tag="vt16")     # chunk-0 k-tiles
            zsum = bigp.tile([DH, S], f32, tag="zsum")

            with tc.tile_pool(name="wts", bufs=1) as wtp, \
                 tc.tile_pool(name="xs", bufs=5) as xsp, \
                 tc.tile_pool(name="et", bufs=6) as etp, \
                 tc.tile_pool(name="rld", bufs=2, space="DRAM") as rldp, \
                 tc.tile_pool(name="lr", bufs=2) as lrp, \
                 tc.tile_pool(name="lbs", bufs=2) as lbsp, \
                 tc.tile_pool(name="zn", bufs=2) as znp, \
                 tc.tile_pool(name="zr", bufs=2) as zrp, \
                 tc.tile_pool(name="osb", bufs=3) as osbp, \
                 tc.tile_pool(name="stp", bufs=3, space="PSUM") as stp, \
                 tc.tile_pool(name="ztp", bufs=2, space="PSUM") as ztp:
                wq8_sb = [wtp.tile([128, 1024], f8, tag=f"wq{k}", name=f"wq{k}")
                          for k in range(4)]
                wk8_sb = [wtp.tile([128, 1024], f8, tag=f"wk{k}", name=f"wk{k}")
                          for k in range(4)]
                wv8_sb = [wtp.tile([128, 1024], f8, tag=f"wv{k}", name=f"wv{k}")
                          for k in range(4)]
                wv16_sb = [wtp.tile([128, GD], f16, tag=f"wv16_{k}",
                                    name=f"wv16_{k}") for k in range(NDT)]
                for k in range(4):
                    nc.scalar.dma_start(wq8_sb[k][:], wq8[k * 128:(k + 1) * 128, :])
                    nc.scalar.dma_start(wk8_sb[k][:], wk8[k * 128:(k + 1) * 128, :])
                    nc.gpsimd.dma_start(wv8_sb[k][:], wv8[k * 128:(k + 1) * 128, :])
                for k in range(NDT):
                    nc.gpsimd.dma_start(wv16_sb[k][:], wv16[k * 128:(k + 1) * 128, :])

                def proj_x(qc):
                    xss = []
                    for st4 in range(4):
                        srow = qc * 512 + st4 * 128
                        xs = xsp.tile([128, D], f16, tag="xs", name="xs")
                        nc.sync.dma_start(xs[:], x[srow:srow + 128, :])
                        xss.append(xs)
                    # transpose x into xT via the PE, 16 per 2-bank PSUM tile;
                    # eviction casts to fp8 (and keeps fp16 for chunk 0)
                    for jj in range(2):
                        pt = stp.tile([128, 2048], f16, tag="st2", name="pt")
                        for j4 in range(4):
                            j = jj * 4 + j4
                            for st4 in range(4):
                                nc.tensor.transpose(
                                    pt[:, j4 * 512 + st4 * 128:
                                       j4 * 512 + (st4 + 1) * 128],
                                    xss[st4][:, j * 128:(j + 1) * 128], idt16[:])
                        dst = bass_ap_3d(xt8_all, (jj * 4) * S + qc * 512, S, 4, 512)
                        nc.vector.tensor_copy(dst, bass_ap_3d(pt, 0, 512, 4, 512))
                        if qc == 0:
                            dst16 = bass_ap_3d(xt16_c0, (jj * 4) * 512, 512, 4, 512)
                            nc.vector.tensor_copy(dst16,
                                                  bass_ap_3d(pt, 0, 512, 4, 512))
                def proj_qk(qc, m):
                    # Q and K m-tile in one PSUM tile: fp8 DoubleRow over d-pairs
                    ps = stp.tile([128, 1024], f32, tag="st2", name="ps")
                    for i, (w_sb, b_t, dest) in enumerate(
                            ((wq8_sb, bq_t, qt_all), (wk8_sb, bk_t, kt_all))):
                        for g in range(4):
                            nc.tensor.matmul(
                                ps[:, i * 512:(i + 1) * 512],
                                ap3(w_sb[g], m * 128, 512, 2, 1, 128),
                                ap3(xt8_all, (2 * g) * S + qc * 512,
                                    S, 2, 1, 512),
                                start=(g == 0), stop=(g == 3),
                                perf_mode=DR)
                        nc.vector.tensor_scalar_add(
                            dest[:, m * S + qc * 512: m * S + (qc + 1) * 512],
                            ps[:, i * 512:(i + 1) * 512],
                            b_t[:, m:m + 1])

                def proj_v(qc, st):
                    stg = qc * 4 + st
                    ps = stp.tile([128, 1024], f32, tag="st2", name="ps")
                    if qc == 0:
                        for k in range(NDT):
                            nc.tensor.matmul(
                                ps[:, 0:512],
                                xt16_c0[:, k * 512 + st * 128:
                                        k * 512 + (st + 1) * 128],
                                wv16_sb[k][:],
                                start=(k == 0), stop=(k == NDT - 1))
                        src = ap3(ps, 0, DH, HPC, 1, DH)
                        nc.vector.tensor_copy(
                            ap3(vt16, st * 520, 65, HPC, 1, DH), src)
                        nc.vector.memset(
                            ap3(vt16, st * 520 + DH, 65, HPC, 1, 1), 1.0)
                    else:
                        for g in range(4):
                            nc.tensor.matmul(
                                ps[:, 0:512],
                                ap3(xt8_all, (2 * g) * S + qc * 512 + st * 128,
                                    S, 2, 1, 128),
                                ap3(wv8_sb[g], 0, 512, 2, 1, 512),
                                start=(g == 0), stop=(g == 3),
                                perf_mode=DR)
                    nc.vector.tensor_copy(
                        ap3(vt8[stg >> 1], (stg & 1) * 640, 80, HPC, 1, DH),
                        ap3(ps, 0, DH, HPC, 1, DH))
                    nc.vector.memset(
                        ap3(vt8[stg >> 1], (stg & 1) * 640 + DH,
                            80, HPC, 1, 1), 1.0)

                def tail_hp(qc, hp, zt0, zt1):
                    lr = lrp.tile([1, 1024], f32, tag="lr")
                    nc.vector.tensor_copy(lr[:, 0:512], zt0[64:65, :])
                    nc.vector.tensor_copy(lr[:, 512:1024], zt1[64:65, :])
                    rr = lrp.tile([1, 1024], f32, tag="lr")
                    nc.vector.reciprocal_approx_fast(rr[:], lr[:])
                    rld = rldp.tile([1, 1024], f32, tag="rld")
                    nc.sync.dma_start(rld[:], rr[:])
                    lbs = lbsp.tile([DH, 1024], f32, tag="lbs")
                    nc.sync.dma_start(lbs[:], _bcast_ap(rld, 0, 0, DH, 1024))
                    zslice = zsum[:, qc * 512:(qc + 1) * 512]
                    for h, zt in ((0, zt0), (1, zt1)):
                        lb = lbs[:, h * 512:(h + 1) * 512]
                        if hp == 0 and h == 0:
                            nc.vector.tensor_tensor(
                                zslice, zt[0:DH, :], lb, op=ALU.mult)
                        else:
                            zn = znp.tile([DH, 512], f32, tag="zn")
                            nc.vector.tensor_tensor(
                                zn[:], zt[0:DH, :], lb, op=ALU.mult)
                            nc.vector.tensor_tensor(
                                zslice, zslice, zn[:], op=ALU.add)

                def attention0():
                    qc = 0
                    for hp in range(4):
                        zt0 = ztp.tile([65, 512], f32, tag="zt", name="zt0")
                        zt1 = ztp.tile([65, 512], f32, tag="zt", name="zt1")
                        for kt in range(4):
                            j = kt
                            jw = j * 128
                            st2 = stp.tile([128, 1024], f32, tag="st2", name="st2")
                            nc.tensor.matmul(
                                st2[:, jw:512],
                                kt_all[0:64, hp * S + kt * 128:
                                       hp * S + (kt + 1) * 128],
                                qt_all[0:64, hp * S + jw: hp * S + 512],
                                start=True, stop=True, tile_position=(0, 0))
                            nc.tensor.matmul(
                                st2[:, 512 + jw:1024],
                                kt_all[64:128, hp * S + kt * 128:
                                       hp * S + (kt + 1) * 128],
                                qt_all[64:128, hp * S + jw: hp * S + 512],
                                start=True, stop=True, tile_position=(64, 0))
                            et = etp.tile([128, 1024], f16, tag="et16", name="et16")
                            nc.scalar.activation(
                                ap3(et, jw, 512, 2, 1, 512 - jw),
                                ap3(st2, jw, 512, 2, 1, 512 - jw),
                                AF.Exp, scale=0.125)
                            for half in range(2):
                                blk = et[:, half * 512 + jw: half * 512 + jw + 128]
                                nc.gpsimd.affine_select(
                                    out=blk, in_=blk, compare_op=ALU.is_ge,
                                    fill=0.0, base=0, pattern=[[1, 128]],
                                    channel_multiplier=-1)
                            nc.tensor.matmul(
                                zt0[:, jw:512],
                                vt16[:, kt * 520 + (2 * hp) * 65:
                                     kt * 520 + (2 * hp) * 65 + 65],
                                et[:, jw:512],
                                start=(kt == 0), stop=(kt == 3))
                            nc.tensor.matmul(
                                zt1[:, jw:512],
                                vt16[:, kt * 520 + (2 * hp + 1) * 65:
                                     kt * 520 + (2 * hp + 1) * 65 + 65],
                                et[:, 512 + jw:1024],
                                start=(kt == 0), stop=(kt == 3))
                        tail_hp(qc, hp, zt0, zt1)

                def attention(qc):
                    npairs = 2 * qc + 2
                    for hp in range(4):
                        zt0 = ztp.tile([65, 512], f32, tag="zt", name="zt0")
                        zt1 = ztp.tile([65, 512], f32, tag="zt", name="zt1")
                        for pi in range(npairs):
                            et = etp.tile([128, 2048], f8, tag="et8", name="et8")
                            jE = 2 * pi - 4 * qc
                            for par in range(2):
                                kt = 2 * pi + par
                                j = kt - 4 * qc
                                jw = max(j, 0) * 128
                                st2 = stp.tile([128, 1024], f32, tag="st2",
                                               name="st2")
                                nc.tensor.matmul(
                                    st2[:, jw:512],
                                    kt_all[0:64, hp * S + kt * 128:
                                           hp * S + (kt + 1) * 128],
                                    qt_all[0:64, hp * S + qc * 512 + jw:
                                           hp * S + qc * 512 + 512],
                                    start=True, stop=True, tile_position=(0, 0))
                                nc.tensor.matmul(
                                    st2[:, 512 + jw:1024],
                                    kt_all[64:128, hp * S + kt * 128:
                                           hp * S + (kt + 1) * 128],
                                    qt_all[64:128, hp * S + qc * 512 + jw:
                                           hp * S + qc * 512 + 512],
                                    start=True, stop=True, tile_position=(64, 0))
                                nc.scalar.activation(
                                    ap3(et, par * 1024 + jw, 512, 2, 1, 512 - jw),
                                    ap3(st2, jw, 512, 2, 1, 512 - jw),
                                    AF.Exp, scale=0.125)
                                if j >= 0:
                                    for half in range(2):
                                        blk = et[:, par * 1024 + half * 512 + jw:
                                                 par * 1024 + half * 512 + jw + 128]
                                        nc.gpsimd.affine_select(
                                            out=blk, in_=blk, compare_op=ALU.is_ge,
                                            fill=0.0, base=0, pattern=[[1, 128]],
                                            channel_multiplier=-1)
                            if jE >= 0:
                                # diagonal pair: even-slot-only delta block as a
                                # plain fp8 matmul, DoubleRow on the shared window
                                d0, d1 = jE * 128, (jE + 1) * 128
                                for h, zt in ((0, zt0), (1, zt1)):
                                    hg = 2 * hp + h
                                    nc.tensor.matmul(
                                        zt[:, d0:d1],
                                        vt8[pi][:, hg * 80: hg * 80 + 65],
                                        et[:, h * 512 + d0: h * 512 + d1],
                                        start=False, stop=False)
                                    nc.tensor.matmul(
                                        zt[:, d1:512],
                                        ap3(vt8[pi], hg * 80, 640, 2, 1, 65),
                                        ap3(et, h * 512 + d1, 1024, 2, 1, 512 - d1),
                                        start=False, stop=(pi == npairs - 1),
                                        perf_mode=DR)
                            else:
                                for h, zt in ((0, zt0), (1, zt1)):
                                    hg = 2 * hp + h
                                    nc.tensor.matmul(
                                        zt[:, 0:512],
                                        ap3(vt8[pi], hg * 80, 640, 2, 1, 65),
                                        ap3(et, h * 512, 1024, 2, 1, 512),
                                        start=(pi == 0), stop=False,
                                        perf_mode=DR)
                        tail_hp(qc, hp, zt0, zt1)

                def tail_proj(qc):
                    zsr = zrp.tile([128, 512], f16, tag="zsr")
                    nc.vector.tensor_copy(zsr[0:DH, :],
                                          zsum[:, qc * 512:(qc + 1) * 512])
                    nc.gpsimd.dma_start(zsr[DH:2 * DH, :],
                                        zsum[:, qc * 512:(qc + 1) * 512])
                    for qp in range(2):
                        for nn in range(2):
                            po = stp.tile([128, 1024], f32, tag="st2", name="po")
                            nc.tensor.matmul(
                                po[:, 0:512],
                                zsr[0:DH, (2 * qp) * 128:(2 * qp + 1) * 128],
                                wo_sb[0:DH, nn * 512:(nn + 1) * 512],
                                start=True, stop=True, tile_position=(0, 0))
                            nc.tensor.matmul(
                                po[:, 512:1024],
                                zsr[DH:128, (2 * qp + 1) * 128:(2 * qp + 2) * 128],
                                wo_sb[DH:128, nn * 512:(nn + 1) * 512],
                                start=True, stop=True, tile_position=(64, 0))
                            osb = osbp.tile([128, 1024], f32, tag="osb")
                            nc.vector.tensor_copy(osb[:], po[:])
                            r0 = qc * 512 + (2 * qp) * 128
                            nc.sync.dma_start(
                                out[r0:r0 + 128, nn * 512:(nn + 1) * 512],
                                osb[:, 0:512])
                            nc.sync.dma_start(
                                out[r0 + 128:r0 + 256, nn * 512:(nn + 1) * 512],
                                osb[:, 512:1024])

                # proj work runs at demoted priority: the list scheduler
                # treats it as filler for PE gaps while attention's
                # ST->exp->ZT chain keeps ScalarE fed.
                for qc in range(NQ):
                    with tc.high_priority(-1000000 * (qc + 1)):
                        proj_x(qc)
                        for m in range(4):
                            proj_qk(qc, m)
                        for st in range(4):
                            proj_v(qc, st)
                    if qc == 0:
                        attention0()
                    else:
                        attention(qc)
                    if qc >= 1:
                        tail_proj(qc - 1)
                tail_proj(NQ - 1)
    nc.compile()
    return nc


def kernel(**inputs):
    import ml_dtypes
    f8 = ml_dtypes.float8_e4m3

    x = np.asarray(inputs["x"], dtype=np.float32)
    WQ = np.asarray(inputs["WQ"], dtype=np.float32)
    bQ = np.asarray(inputs["bQ"], dtype=np.float32)
    WK = np.asarray(inputs["WK"], dtype=np.float32)
    bK = np.asarray(inputs["bK"], dtype=np.float32)
    WV = np.asarray(inputs["WV"], dtype=np.float32)
    bV = np.asarray(inputs["bV"], dtype=np.float32)
    WO = np.asarray(inputs["WO"], dtype=np.float32)
    bO = np.asarray(inputs["bO"], dtype=np.float32)

    from concourse.bass_utils import run_bass_kernel_spmd

    if "nc" not in _prog:
        _prog["nc"] = _build()
    nc = _prog["nc"]

    def pair_layout(Wc):
        # [1024, 512] -> [512, 1024]: out[g*128+p, par*512+m] = Wc[g*256+par*128+p, m]
        return np.ascontiguousarray(
            Wc.reshape(4, 2, 128, GD).transpose(0, 2, 1, 3).reshape(512, 1024))

    in_maps = []
    for c in range(NCORES):
        b, g = c // 2, c % 2
        sl = slice(g * GD, (g + 1) * GD)
        xb = np.ascontiguousarray(x[b])
        in_maps.append({
            "x": xb.astype(np.float16),
            "wq8": pair_layout(WQ[:, sl]).astype(f8),
            "wk8": pair_layout(WK[:, sl]).astype(f8),
            "wv8": pair_layout(WV[:, sl]).astype(f8),
            "wv16": np.ascontiguousarray(WV[:, sl]).astype(np.float16),
            "bq": np.ascontiguousarray(bQ[sl]).reshape(1, GD).astype(np.float16),
            "bk": np.ascontiguousarray(bK[sl]).reshape(1, GD).astype(np.float16),
            "wo": WO.astype(np.float16),
        })
    _prog["in_maps"] = in_maps
    res = run_bass_kernel_spmd(nc, in_maps, core_ids=list(range(NCORES)))
    parts = [r["out"] for r in res.results]

    extra = bV.reshape(H, DH).sum(0) @ WO + np.float32(H) * bO
    out = np.empty((B, S, D), dtype=np.float32)
    for b in range(B):
        out[b] = parts[2 * b] + parts[2 * b + 1] + extra
    return out


# revision 13
# speedup vs baseline: 1.1182x; 1.1182x over previous
"""Causal self-attention kernel for 8 Trainium2 NeuronCores.

Sharding: core c -> (batch b = c//2, head-group g = c%2). Each core computes
the attention output contribution of 8 heads for one batch element:
    P_c = (sum_{h in group} softmax(Q_h K_h^T / 8 + causal) V_h) @ WO
Host epilogue: out[b] = P_{2b} + P_{2b+1} + (sum_h bV_h) @ WO + 16*bO
(the V-bias commutes through softmax normalization: softmax rows sum to 1).

Precision/speed split (PE row counts at ~1.5 G rows/s are the limiter):
  - Q/K/V projections run as fp8-E4M3 DoubleRow matmuls (K=256 per pass:
    d-tile pairs via a [128, 2, N] AP over the existing xt layout), halving
    projection PE time. Chunk 0's V is also computed in fp16 (kept in vt16)
    because output rows with tiny softmax support (q < 512) see V error
    directly; all other chunks' softmax averages 512+ values so fp8 noise
    washes out.
  - Scores stay fp16 (K=64 contraction gets no DoubleRow benefit); two heads
    per PE pass via tile_position row-tiling. Q/K fp8 projection error only
    perturbs softmax weights (benign at any support size).
  - A@V for q-chunks >= 1 runs fp8 DoubleRow over k-tile PAIRS: ET pair
    tiles [128, 2, 2head*512] written by exp directly in fp8, V pair tiles
    [128, 2, 8head*65] with a ones column accumulating the softmax
    denominator in row 64. Chunk 0 uses the fp16 per-k-tile path.
  - Diagonal k-tiles shrink the ST matmul, exp, and ZT to the unmasked
    q-window; fully masked regions are never computed (no memsets except the
    odd-slot delta of diagonal fp8 pairs). The in-block triangle is masked
    by gpsimd affine_select on ET after exp.
  - Per-hp tail: 1/l via DVE reciprocal_approx_fast (no ScalarE Ln/Exp table
    swaps), broadcast by a DRAM-bounce DMA, normalize straight out of zt
    PSUM into the zsum accumulator, so almost nothing serializes at the end.
"""
import numpy as np

B, S, D, H, DH = 4, 2048, 1024, 16, 64
HPC = 8            # heads per core
GD = HPC * DH      # 512 = group width
NCORES = 8
NQ = S // 512      # 4 q/s chunks of 512
NKT = S // 128     # 16 k-tiles
NDT = D // 128     # 8 d-tiles

_prog = {}


def ap3(tile_t, offset, d1, n1, d2, n2):
    """AP view [128p, n1, n2] over a tile's free dim: col = offset + i*d1 + j*d2."""
    import concourse.bass as bass
    ap = tile_t[:]
    return bass.AP(ap.tensor, ap.offset + offset,
                   [ap.ap[0], [d1, n1], [d2, n2]])


def bass_ap_3d(tile_t, offset, stride, n, inner):
    return ap3(tile_t, offset, stride, n, 1, inner)


def _bcast_ap(tile_t, row, col, nparts, width):
    """Partition-step-0 AP reading (row, col:col+width) replicated nparts times."""
    import concourse.bass as bass
    ap = tile_t[:]
    pstep = ap.ap[0][0]
    return bass.AP(ap.tensor, ap.offset + row * pstep + col,
                   [[0, nparts], [1, width]])


def _build():
    import concourse.bacc as bacc
    import concourse.tile as tile
    from concourse import mybir
    import concourse.bass as bass

    f32 = mybir.dt.float32
    f16 = mybir.dt.float16
    f8 = mybir.dt.float8e4
    AF = mybir.ActivationFunctionType
    ALU = mybir.AluOpType
    DR = mybir.MatmulPerfMode.DoubleRow

    nc = bacc.Bacc(None, target_bir_lowering=False, debug=False)
    x = nc.dram_tensor("x", [S, D], f16, kind="ExternalInput")
    # wq8/wk8/wv8: [4*128, 2*512]: row = g*128 + p, col = par*512 + m,
    # value = W[g*256 + par*128 + p, m]  (d-pair DoubleRow layout)
    wq8 = nc.dram_tensor("wq8", [512, 1024], f8, kind="ExternalInput")
    wk8 = nc.dram_tensor("wk8", [512, 1024], f8, kind="ExternalInput")
    wv8 = nc.dram_tensor("wv8", [512, 1024], f8, kind="ExternalInput")
    wv16 = nc.dram_tensor("wv16", [D, GD], f16, kind="ExternalInput")
    bq = nc.dram_tensor("bq", [1, GD], f16, kind="ExternalInput")
    bk = nc.dram_tensor("bk", [1, GD], f16, kind="ExternalInput")
    wo = nc.dram_tensor("wo", [DH, D], f16, kind="ExternalInput")
    out = nc.dram_tensor("out", [S, D], f32, kind="ExternalOutput")

    with tile.TileContext(nc) as tc:
        with tc.tile_pool(name="const", bufs=1) as constp, \
             tc.tile_pool(name="big", bufs=1) as bigp:
            idt16 = constp.tile([128, 128], f16, tag="idt16")
            from concourse.masks import make_identity
            make_identity(nc, idt16[:])
            bq_t = constp.tile([128, 4], f32, tag="bq_t")
            bk_t = constp.tile([128, 4], f32, tag="bk_t")
            nc.gpsimd.dma_start(bq_t[:], bass.AP(bq, 0, [[1, 128], [128, 4]]))
            nc.gpsimd.dma_start(bk_t[:], bass.AP(bk, 0, [[1, 128], [128, 4]]))
            wo_sb = constp.tile([128, D], f16, tag="wo_sb")
            nc.gpsimd.dma_start(wo_sb[0:DH, :], wo[:])
            nc.gpsimd.dma_start(wo_sb[DH:2 * DH, :], wo[:])

            # persistent per-core tensors
            xt8_all = bigp.tile([128, NDT * S], f8, tag="xt8")    # d-tile j at j*S
            xt16_c0 = bigp.tile([128, NDT * 512], f16, tag="xt16")  # d-tile j at j*512
            qt_all = bigp.tile([128, 4 * S], f16, tag="qt")       # m-tile m at m*S
            kt_all = bigp.tile([128, 4 * S], f16, tag="kt")
            # vt8: k-tile pair p: [par(2) x head(8) x 65] at col par*520+h*65+c
            vt8 = [bigp.tile([128, 1280], f8, tag=f"vt8_{p}", name=f"vt8_{p}")
                   for p in range(NKT // 2)]
            vt16 = bigp.tile([128, 4 * 520], f16, tag="vt16")     # chunk-0 k-tiles
            zsum = bigp.tile([DH, S], f32, tag="zsum")

            with tc.tile_pool(name="wts", bufs=1) as wtp, \
                 tc.tile_pool(name="xs", bufs=5) as xsp, \
                 tc.tile_pool(name="et", bufs=8) as etp, \
                 tc.tile_pool(name="rld", bufs=2, space="DRAM") as rldp, \
                 tc.tile_pool(name="lr", bufs=2) as lrp, \
                 tc.tile_pool(name="lbs", bufs=2) as lbsp, \
                 tc.tile_pool(name="zn", bufs=2) as znp, \
                 tc.tile_pool(name="zr", bufs=2) as zrp, \
                 tc.tile_pool(name="osb", bufs=3) as osbp, \
                 tc.tile_pool(name="stp", bufs=3, space="PSUM") as stp, \
                 tc.tile_pool(name="ztp", bufs=2, space="PSUM") as ztp:
                wq8_sb = [wtp.tile([128, 1024], f8, tag=f"wq{k}", name=f"wq{k}")
                          for k in range(4)]
                wk8_sb = [wtp.tile([128, 1024], f8, tag=f"wk{k}", name=f"wk{k}")
                          for k in range(4)]
                wv8_sb = [wtp.tile([128, 1024], f8, tag=f"wv{k}", name=f"wv{k}")
                          for k in range(4)]
                wv16_sb = [wtp.tile([128, GD], f16, tag=f"wv16_{k}",
                                    name=f"wv16_{k}") for k in range(NDT)]
                for k in range(4):
                    nc.scalar.dma_start(wq8_sb[k][:], wq8[k * 128:(k + 1) * 128, :])
                    nc.scalar.dma_start(wk8_sb[k][:], wk8[k * 128:(k + 1) * 128, :])
                    nc.gpsimd.dma_start(wv8_sb[k][:], wv8[k * 128:(k + 1) * 128, :])
                for k in range(NDT):
                    nc.gpsimd.dma_start(wv16_sb[k][:], wv16[k * 128:(k + 1) * 128, :])

                def proj_x(qc):
                    xss = []
                    for st4 in range(4):
                        srow = qc * 512 + st4 * 128
                        xs = xsp.tile([128, D], f16, tag="xs", name="xs")
                        nc.sync.dma_start(xs[:], x[srow:srow + 128, :])
                        xss.append(xs)
                    # transpose x into xT via the PE, 16 per 2-bank PSUM tile;
                    # eviction casts to fp8 (and keeps fp16 for chunk 0)
                    for jj in range(2):
                        pt = stp.tile([128, 2048], f16, tag="st2", name="pt")
                        for j4 in range(4):
                            j = jj * 4 + j4
                            for st4 in range(4):
                                nc.tensor.transpose(
                                    pt[:, j4 * 512 + st4 * 128:
                                       j4 * 512 + (st4 + 1) * 128],
                                    xss[st4][:, j * 128:(j + 1) * 128], idt16[:])
                        dst = bass_ap_3d(xt8_all, (jj * 4) * S + qc * 512, S, 4, 512)
                        nc.vector.tensor_copy(dst, bass_ap_3d(pt, 0, 512, 4, 512))
                        if qc == 0:
                            dst16 = bass_ap_3d(xt16_c0, (jj * 4) * 512, 512, 4, 512)
                            nc.vector.tensor_copy(dst16,
                                                  bass_ap_3d(pt, 0, 512, 4, 512))
                def proj_qk(qc, m):
                    # Q and K m-tile in one PSUM tile: fp8 DoubleRow over d-pairs
                    ps = stp.tile([128, 1024], f32, tag="st2", name="ps")
                    for i, (w_sb, b_t, dest) in enumerate(
                            ((wq8_sb, bq_t, qt_all), (wk8_sb, bk_t, kt_all))):
                        for g in range(4):
                            nc.tensor.matmul(
                                ps[:, i * 512:(i + 1) * 512],
                                ap3(w_sb[g], m * 128, 512, 2, 1, 128),
                                ap3(xt8_all, (2 * g) * S + qc * 512,
                                    S, 2, 1, 512),
                                start=(g == 0), stop=(g == 3),
                                perf_mode=DR)
                        nc.vector.tensor_scalar_add(
                            dest[:, m * S + qc * 512: m * S + (qc + 1) * 512],
                            ps[:, i * 512:(i + 1) * 512],
                            b_t[:, m:m + 1])

                def proj_v(qc, st):
                    stg = qc * 4 + st
                    ps = stp.tile([128, 1024], f32, tag="st2", name="ps")
                    if qc == 0:
                        for k in range(NDT):
                            nc.tensor.matmul(
                                ps[:, 0:512],
                                xt16_c0[:, k * 512 + st * 128:
                                        k * 512 + (st + 1) * 128],
                                wv16_sb[k][:],
                                start=(k == 0), stop=(k == NDT - 1))
                        src = ap3(ps, 0, DH, HPC, 1, DH)
                        nc.vector.tensor_copy(
                            ap3(vt16, st * 520, 65, HPC, 1, DH), src)
                        nc.vector.memset(
                            ap3(vt16, st * 520 + DH, 65, HPC, 1, 1), 1.0)
                    else:
                        for g in range(4):
                            nc.tensor.matmul(
                                ps[:, 0:512],
                                ap3(xt8_all, (2 * g) * S + qc * 512 + st * 128,
                                    S, 2, 1, 128),
                                ap3(wv8_sb[g], 0, 512, 2, 1, 512),
                                start=(g == 0), stop=(g == 3),
                                perf_mode=DR)
                    nc.vector.tensor_copy(
                        ap3(vt8[stg >> 1], (stg & 1) * 640, 80, HPC, 1, DH),
                        ap3(ps, 0, DH, HPC, 1, DH))
                    nc.vector.memset(
                        ap3(vt8[stg >> 1], (stg & 1) * 640 + DH,
                            80, HPC, 1, 1), 1.0)

                def tail_hp(qc, hp, zt0, zt1):
                    lr = lrp.tile([1, 1024], f32, tag="lr")
                    nc.vector.tensor_copy(lr[:, 0:512], zt0[64:65, :])
                    nc.vector.tensor_copy(lr[:, 512:1024], zt1[64:65, :])
                    rr = lrp.tile([1, 1024], f32, tag="lr")
                    nc.vector.reciprocal_approx_fast(rr[:], lr[:])
                    rld = rldp.tile([1, 1024], f32, tag="rld")
                    nc.sync.dma_start(rld[:], rr[:])
                    lbs = lbsp.tile([DH, 1024], f32, tag="lbs")
                    nc.sync.dma_start(lbs[:], _bcast_ap(rld, 0, 0, DH, 1024))
                    zslice = zsum[:, qc * 512:(qc + 1) * 512]
                    for h, zt in ((0, zt0), (1, zt1)):
                        lb = lbs[:, h * 512:(h + 1) * 512]
                        if hp == 0 and h == 0:
                            nc.vector.tensor_tensor(
                                zslice, zt[0:DH, :], lb, op=ALU.mult)
                        else:
                            zn = znp.tile([DH, 512], f32, tag="zn")
                            nc.vector.tensor_tensor(
                                zn[:], zt[0:DH, :], lb, op=ALU.mult)
                            nc.vector.tensor_tensor(
                                zslice, zslice, zn[:], op=ALU.add)

                def attention0():
                    qc = 0
                    for hp in range(4):
                        zt0 = ztp.tile([65, 512], f32, tag="zt", name="zt0")
                        zt1 = ztp.tile([65, 512], f32, tag="zt", name="zt1")
                        for kt in range(4):
                            j = kt
                            jw = j * 128
                            st2 = stp.tile([128, 1024], f32, tag="st2", name="st2")
                            nc.tensor.matmul(
                                st2[:, jw:512],
                                kt_all[0:64, hp * S + kt * 128:
                                       hp * S + (kt + 1) * 128],
                                qt_all[0:64, hp * S + jw: hp * S + 512],
                                start=True, stop=True, tile_position=(0, 0))
                            nc.tensor.matmul(
                                st2[:, 512 + jw:1024],
                                kt_all[64:128, hp * S + kt * 128:
                                       hp * S + (kt + 1) * 128],
                                qt_all[64:128, hp * S + jw: hp * S + 512],
                                start=True, stop=True, tile_position=(64, 0))
                            et = etp.tile([128, 1024], f16, tag="et16", name="et16")
                            nc.scalar.activation(
                                ap3(et, jw, 512, 2, 1, 512 - jw),
                                ap3(st2, jw, 512, 2, 1, 512 - jw),
                                AF.Exp, scale=0.125)
                            for half in range(2):
                                blk = et[:, half * 512 + jw: half * 512 + jw + 128]
                                nc.gpsimd.affine_select(
                                    out=blk, in_=blk, compare_op=ALU.is_ge,
                                    fill=0.0, base=0, pattern=[[1, 128]],
                                    channel_multiplier=-1)
                            nc.tensor.matmul(
                                zt0[:, jw:512],
                                vt16[:, kt * 520 + (2 * hp) * 65:
                                     kt * 520 + (2 * hp) * 65 + 65],
                                et[:, jw:512],
                                start=(kt == 0), stop=(kt == 3))
                            nc.tensor.matmul(
                                zt1[:, jw:512],
                                vt16[:, kt * 520 + (2 * hp + 1) * 65:
                                     kt * 520 + (2 * hp + 1) * 65 + 65],
                                et[:, 512 + jw:1024],
                                start=(kt == 0), stop=(kt == 3))
                        tail_hp(qc, hp, zt0, zt1)

                def attention(qc):
                    npairs = 2 * qc + 2
                    for hp in range(4):
                        zt0 = ztp.tile([65, 512], f32, tag="zt", name="zt0")
                        zt1 = ztp.tile([65, 512], f32, tag="zt", name="zt1")
                        for pi in range(npairs):
                            et = etp.tile([128, 2048], f8, tag="et8", name="et8")
                            jE = 2 * pi - 4 * qc
                            for par in range(2):
                                kt = 2 * pi + par
                                j = kt - 4 * qc
                                jw = max(j, 0) * 128
                                st2 = stp.tile([128, 1024], f32, tag="st2",
                                               name="st2")
                                nc.tensor.matmul(
                                    st2[:, jw:512],
                                    kt_all[0:64, hp * S + kt * 128:
                                           hp * S + (kt + 1) * 128],
                                    qt_all[0:64, hp * S + qc * 512 + jw:
                                           hp * S + qc * 512 + 512],
                                    start=True, stop=True, tile_position=(0, 0))
                                nc.tensor.matmul(
                                    st2[:, 512 + jw:1024],
                                    kt_all[64:128, hp * S + kt * 128:
                                           hp * S + (kt + 1) * 128],
                                    qt_all[64:128, hp * S + qc * 512 + jw:
                                           hp * S + qc * 512 + 512],
                                    start=True, stop=True, tile_position=(64, 0))
                                nc.scalar.activation(
                                    ap3(et, par * 1024 + jw, 512, 2, 1, 512 - jw),
                                    ap3(st2, jw, 512, 2, 1, 512 - jw),
                                    AF.Exp, scale=0.125)
                                if j >= 0:
                                    for half in range(2):
                                        blk = et[:, par * 1024 + half * 512 + jw:
                                                 par * 1024 + half * 512 + jw + 128]
                                        nc.gpsimd.affine_select(
                                            out=blk, in_=blk, compare_op=ALU.is_ge,
                                            fill=0.0, base=0, pattern=[[1, 128]],
                                            channel_multiplier=-1)
                            if jE >= 0:
                                # diagonal pair: even-slot-only delta block as a
                                # plain fp8 matmul, DoubleRow on the shared window
                                d0, d1 = jE * 128, (jE + 1) * 128
                                for h, zt in ((0, zt0), (1, zt1)):
                                    hg = 2 * hp + h
                                    nc.tensor.matmul(
                                        zt[:, d0:d1],
                                        vt8[pi][:, hg * 80: hg * 80 + 65],
                                        et[:, h * 512 + d0: h * 512 + d1],
                                        start=False, stop=False)
                                    nc.tensor.matmul(
                                        zt[:, d1:512],
                                        ap3(vt8[pi], hg * 80, 640, 2, 1, 65),
                                        ap3(et, h * 512 + d1, 1024, 2, 1, 512 - d1),
                                        start=False, stop=(pi == npairs - 1),
                                        perf_mode=DR)
                            else:
                                for h, zt in ((0, zt0), (1, zt1)):
                                    hg = 2 * hp + h
                                    nc.tensor.matmul(
                                        zt[:, 0:512],
                                        ap3(vt8[pi], hg * 80, 640, 2, 1, 65),
                                        ap3(et, h * 512, 1024, 2, 1, 512),
                                        start=(pi == 0), stop=False,
                                        perf_mode=DR)
                        tail_hp(qc, hp, zt0, zt1)

                def tail_proj(qc):
                    zsr = zrp.tile([128, 512], f16, tag="zsr")
                    nc.vector.tensor_copy(zsr[0:DH, :],
                                          zsum[:, qc * 512:(qc + 1) * 512])
                    nc.gpsimd.dma_start(zsr[DH:2 * DH, :],
                                        zsum[:, qc * 512:(qc + 1) * 512])
                    for qp in range(2):
                        for nn in range(2):
                            po = stp.tile([128, 1024], f32, tag="st2", name="po")
                            nc.tensor.matmul(
                                po[:, 0:512],
                                zsr[0:DH, (2 * qp) * 128:(2 * qp + 1) * 128],
                                wo_sb[0:DH, nn * 512:(nn + 1) * 512],
                                start=True, stop=True, tile_position=(0, 0))
                            nc.tensor.matmul(
                                po[:, 512:1024],
                                zsr[DH:128, (2 * qp + 1) * 128:(2 * qp + 2) * 128],
                                wo_sb[DH:128, nn * 512:(nn + 1) * 512],
                                start=True, stop=True, tile_position=(64, 0))
                            osb = osbp.tile([128, 1024], f32, tag="osb")
                            nc.vector.tensor_copy(osb[:], po[:])
                            r0 = qc * 512 + (2 * qp) * 128
                            nc.sync.dma_start(
                                out[r0:r0 + 128, nn * 512:(nn + 1) * 512],
                                osb[:, 0:512])
                            nc.sync.dma_start(
                                out[r0 + 128:r0 + 256, nn * 512:(nn + 1) * 512],
                                osb[:, 512:1024])

                for qc in range(NQ):
                    proj_x(qc)
                    for m in range(4):
                        proj_qk(qc, m)
                    for st in range(4):
                        proj_v(qc, st)
                    if qc == 0:
                        attention0()
                    else:
                        attention(qc)
                    if qc >= 1:
                        tail_proj(qc - 1)
                tail_proj(NQ - 1)
    nc.compile()
    return nc


def kernel(**inputs):
    import ml_dtypes
    f8 = ml_dtypes.float8_e4m3

    x = np.asarray(inputs["x"], dtype=np.float32)
    WQ = np.asarray(inputs["WQ"], dtype=np.float32)
    bQ = np.asarray(inputs["bQ"], dtype=np.float32)
    WK = np.asarray(inputs["WK"], dtype=np.float32)
    bK = np.asarray(inputs["bK"], dtype=np.float32)
    WV = np.asarray(inputs["WV"], dtype=np.float32)
    bV = np.asarray(inputs["bV"], dtype=np.float32)
    WO = np.asarray(inputs["WO"], dtype=np.float32)
    bO = np.asarray(inputs["bO"], dtype=np.float32)

    from concourse.bass_utils import run_bass_kernel_spmd

    if "nc" not in _prog:
        _prog["nc"] = _build()
    nc = _prog["nc"]

    def pair_layout(Wc):
        # [1024, 512] -> [512, 1024]: out[g*128+p, par*512+m] = Wc[g*256+par*128+p, m]
        return np.ascontiguousarray(
            Wc.reshape(4, 2, 128, GD).transpose(0, 2, 1, 3).reshape(512, 1024))

    in_maps = []
    for c in range(NCORES):
        b, g = c // 2, c % 2
        sl = slice(g * GD, (g + 1) * GD)
        xb = np.ascontiguousarray(x[b])
        in_maps.append({
            "x": xb.astype(np.float16),
            "wq8": pair_layout(WQ[:, sl]).astype(f8),
            "wk8": pair_layout(WK[:, sl]).astype(f8),
            "wv8": pair_layout(WV[:, sl]).astype(f8),
            "wv16": np.ascontiguousarray(WV[:, sl]).astype(np.float16),
            "bq": np.ascontiguousarray(bQ[sl]).reshape(1, GD).astype(np.float16),
            "bk": np.ascontiguousarray(bK[sl]).reshape(1, GD).astype(np.float16),
            "wo": WO.astype(np.float16),
        })
    _prog["in_maps"] = in_maps
    res = run_bass_kernel_spmd(nc, in_maps, core_ids=list(range(NCORES)))
    parts = [r["out"] for r in res.results]

    extra = bV.reshape(H, DH).sum(0) @ WO + np.float32(H) * bO
    out = np.empty((B, S, D), dtype=np.float32)
    for b in range(B):
        out[b] = parts[2 * b] + parts[2 * b + 1] + extra
    return out


# revision 14
# speedup vs baseline: 1.2313x; 1.1012x over previous
"""Causal self-attention kernel for 8 Trainium2 NeuronCores.

Sharding: core c -> (batch b = c//2, head-group g = c%2). Each core computes
the attention output contribution of 8 heads for one batch element:
    P_c = (sum_{h in group} softmax(Q_h K_h^T / 8 + causal) V_h) @ WO
Host epilogue: out[b] = P_{2b} + P_{2b+1} + (sum_h bV_h) @ WO + 16*bO
(the V-bias commutes through softmax normalization: softmax rows sum to 1).

Precision/speed split (PE row counts at ~1.5 G rows/s are the limiter):
  - Q/K/V projections run as fp8-E4M3 DoubleRow matmuls (K=256 per pass:
    d-tile pairs via a [128, 2, N] AP over the existing xt layout), halving
    projection PE time. Chunk 0's V is also computed in fp16 (kept in vt16)
    because output rows with tiny softmax support (q < 512) see V error
    directly; all other chunks' softmax averages 512+ values so fp8 noise
    washes out.
  - Scores stay fp16 (K=64 contraction gets no DoubleRow benefit); two heads
    per PE pass via tile_position row-tiling. Q/K fp8 projection error only
    perturbs softmax weights (benign at any support size).
  - A@V for q-chunks >= 1 runs fp8 DoubleRow over k-tile PAIRS: ET pair
    tiles [128, 2, 2head*512] written by exp directly in fp8, V pair tiles
    [128, 2, 8head*65] with a ones column accumulating the softmax
    denominator in row 64. Chunk 0 uses the fp16 per-k-tile path.
  - Diagonal k-tiles shrink the ST matmul, exp, and ZT to the unmasked
    q-window; fully masked regions are never computed (no memsets except the
    odd-slot delta of diagonal fp8 pairs). The in-block triangle is masked
    by gpsimd affine_select on ET after exp.
  - Per-hp tail: 1/l via DVE reciprocal_approx_fast (no ScalarE Ln/Exp table
    swaps), broadcast by a DRAM-bounce DMA, normalize straight out of zt
    PSUM into the zsum accumulator, so almost nothing serializes at the end.
"""
import numpy as np

B, S, D, H, DH = 4, 2048, 1024, 16, 64
HPC = 8            # heads per core
GD = HPC * DH      # 512 = group width
NCORES = 8
NQ = S // 512      # 4 q/s chunks of 512
NKT = S // 128     # 16 k-tiles
NDT = D // 128     # 8 d-tiles

_prog = {}


def ap3(tile_t, offset, d1, n1, d2, n2):
    """AP view [128p, n1, n2] over a tile's free dim: col = offset + i*d1 + j*d2."""
    import concourse.bass as bass
    ap = tile_t[:]
    return bass.AP(ap.tensor, ap.offset + offset,
                   [ap.ap[0], [d1, n1], [d2, n2]])


def bass_ap_3d(tile_t, offset, stride, n, inner):
    return ap3(tile_t, offset, stride, n, 1, inner)


def _bcast_ap(tile_t, row, col, nparts, width):
    """Partition-step-0 AP reading (row, col:col+width) replicated nparts times."""
    import concourse.bass as bass
    ap = tile_t[:]
    pstep = ap.ap[0][0]
    return bass.AP(ap.tensor, ap.offset + row * pstep + col,
                   [[0, nparts], [1, width]])


def _build():
    import concourse.bacc as bacc
    import concourse.tile as tile
    from concourse import mybir
    import concourse.bass as bass

    f32 = mybir.dt.float32
    f16 = mybir.dt.float16
    f8 = mybir.dt.float8e4
    AF = mybir.ActivationFunctionType
    ALU = mybir.AluOpType
    DR = mybir.MatmulPerfMode.DoubleRow

    nc = bacc.Bacc(None, target_bir_lowering=False, debug=False)
    x = nc.dram_tensor("x", [S, D], f16, kind="ExternalInput")
    # wq8/wk8/wv8: [4*128, 2*512]: row = g*128 + p, col = par*512 + m,
    # value = W[g*256 + par*128 + p, m]  (d-pair DoubleRow layout)
    wq8 = nc.dram_tensor("wq8", [512, 1024], f8, kind="ExternalInput")
    wk8 = nc.dram_tensor("wk8", [512, 1024], f8, kind="ExternalInput")
    wv8 = nc.dram_tensor("wv8", [512, 1024], f8, kind="ExternalInput")
    wv16 = nc.dram_tensor("wv16", [D, GD], f16, kind="ExternalInput")
    bq = nc.dram_tensor("bq", [1, GD], f16, kind="ExternalInput")
    bk = nc.dram_tensor("bk", [1, GD], f16, kind="ExternalInput")
    wo = nc.dram_tensor("wo", [DH, D], f16, kind="ExternalInput")
    out = nc.dram_tensor("out", [S, D], f32, kind="ExternalOutput")

    with tile.TileContext(nc) as tc:
        with tc.tile_pool(name="const", bufs=1) as constp, \
             tc.tile_pool(name="big", bufs=1) as bigp:
            idt16 = constp.tile([128, 128], f16, tag="idt16")
            from concourse.masks import make_identity
            make_identity(nc, idt16[:])
            bq_t = constp.tile([128, 4], f32, tag="bq_t")
            bk_t = constp.tile([128, 4], f32, tag="bk_t")
            nc.gpsimd.dma_start(bq_t[:], bass.AP(bq, 0, [[1, 128], [128, 4]]))
            nc.gpsimd.dma_start(bk_t[:], bass.AP(bk, 0, [[1, 128], [128, 4]]))
            wo_sb = constp.tile([128, D], f16, tag="wo_sb")
            nc.gpsimd.dma_start(wo_sb[0:DH, :], wo[:])
            nc.gpsimd.dma_start(wo_sb[DH:2 * DH, :], wo[:])

            # persistent per-core tensors
            xt8_all = bigp.tile([128, NDT * S], f8, tag="xt8")    # d-tile j at j*S
            xt16_c0 = bigp.tile([128, NDT * 512], f16, tag="xt16")  # d-tile j at j*512
            qt_all = bigp.tile([128, 4 * S], f16, tag="qt")       # m-tile m at m*S
            kt_all = bigp.tile([128, 4 * S], f16, tag="kt")
            # vt8: k-tile pair p: [par(2) x head(8) x 65] at col par*520+h*65+c
            vt8 = [bigp.tile([128, 1280], f8, tag=f"vt8_{p}", name=f"vt8_{p}")
                   for p in range(NKT // 2)]
            vt16 = bigp.tile([128, 4 * 520], f16, tag="vt16")     # chunk-0 k-tiles
            zsum = bigp.tile([DH, S], f32, tag="zsum")

            with tc.tile_pool(name="wts", bufs=1) as wtp, \
                 tc.tile_pool(name="xs", bufs=5) as xsp, \
                 tc.tile_pool(name="et16", bufs=6) as etp16, \
                 tc.tile_pool(name="et8", bufs=16) as etp8, \
                 tc.tile_pool(name="rld", bufs=2, space="DRAM") as rldp, \
                 tc.tile_pool(name="lr", bufs=2) as lrp, \
                 tc.tile_pool(name="lbs", bufs=2) as lbsp, \
                 tc.tile_pool(name="zn", bufs=2) as znp, \
                 tc.tile_pool(name="zr", bufs=2) as zrp, \
                 tc.tile_pool(name="osb", bufs=3) as osbp, \
                 tc.tile_pool(name="stp", bufs=3, space="PSUM") as stp, \
                 tc.tile_pool(name="ztp", bufs=2, space="PSUM") as ztp:
                wq8_sb = [wtp.tile([128, 1024], f8, tag=f"wq{k}", name=f"wq{k}")
                          for k in range(4)]
                wk8_sb = [wtp.tile([128, 1024], f8, tag=f"wk{k}", name=f"wk{k}")
                          for k in range(4)]
                wv8_sb = [wtp.tile([128, 1024], f8, tag=f"wv{k}", name=f"wv{k}")
                          for k in range(4)]
                wv16_sb = [wtp.tile([128, GD], f16, tag=f"wv16_{k}",
                                    name=f"wv16_{k}") for k in range(NDT)]
                for k in range(4):
                    nc.scalar.dma_start(wq8_sb[k][:], wq8[k * 128:(k + 1) * 128, :])
                    nc.scalar.dma_start(wk8_sb[k][:], wk8[k * 128:(k + 1) * 128, :])
                    nc.gpsimd.dma_start(wv8_sb[k][:], wv8[k * 128:(k + 1) * 128, :])
                for k in range(NDT):
                    nc.gpsimd.dma_start(wv16_sb[k][:], wv16[k * 128:(k + 1) * 128, :])

                def proj_x(qc):
                    xss = []
                    for st4 in range(4):
                        srow = qc * 512 + st4 * 128
                        xs = xsp.tile([128, D], f16, tag="xs", name="xs")
                        nc.sync.dma_start(xs[:], x[srow:srow + 128, :])
                        xss.append(xs)
                    # transpose x into xT via the PE, 16 per 2-bank PSUM tile;
                    # eviction casts to fp8 (and keeps fp16 for chunk 0)
                    for jj in range(2):
                        pt = stp.tile([128, 2048], f16, tag="st2", name="pt")
                        for j4 in range(4):
                            j = jj * 4 + j4
                            for st4 in range(4):
                                nc.tensor.transpose(
                                    pt[:, j4 * 512 + st4 * 128:
                                       j4 * 512 + (st4 + 1) * 128],
                                    xss[st4][:, j * 128:(j + 1) * 128], idt16[:])
                        dst = bass_ap_3d(xt8_all, (jj * 4) * S + qc * 512, S, 4, 512)
                        nc.vector.tensor_copy(dst, bass_ap_3d(pt, 0, 512, 4, 512))
                        if qc == 0:
                            dst16 = bass_ap_3d(xt16_c0, (jj * 4) * 512, 512, 4, 512)
                            nc.vector.tensor_copy(dst16,
                                                  bass_ap_3d(pt, 0, 512, 4, 512))
                def proj_qk(qc, m):
                    # Q and K m-tile in one PSUM tile: fp8 DoubleRow over d-pairs
                    ps = stp.tile([128, 1024], f32, tag="st2", name="ps")
                    for i, (w_sb, b_t, dest) in enumerate(
                            ((wq8_sb, bq_t, qt_all), (wk8_sb, bk_t, kt_all))):
                        for g in range(4):
                            nc.tensor.matmul(
                                ps[:, i * 512:(i + 1) * 512],
                                ap3(w_sb[g], m * 128, 512, 2, 1, 128),
                                ap3(xt8_all, (2 * g) * S + qc * 512,
                                    S, 2, 1, 512),
                                start=(g == 0), stop=(g == 3),
                                perf_mode=DR)
                        nc.vector.tensor_scalar_add(
                            dest[:, m * S + qc * 512: m * S + (qc + 1) * 512],
                            ps[:, i * 512:(i + 1) * 512],
                            b_t[:, m:m + 1])

                def proj_v(qc, st):
                    stg = qc * 4 + st
                    ps = stp.tile([128, 1024], f32, tag="st2", name="ps")
                    if qc == 0:
                        for k in range(NDT):
                            nc.tensor.matmul(
                                ps[:, 0:512],
                                xt16_c0[:, k * 512 + st * 128:
                                        k * 512 + (st + 1) * 128],
                                wv16_sb[k][:],
                                start=(k == 0), stop=(k == NDT - 1))
                        src = ap3(ps, 0, DH, HPC, 1, DH)
                        nc.vector.tensor_copy(
                            ap3(vt16, st * 520, 65, HPC, 1, DH), src)
                        nc.vector.memset(
                            ap3(vt16, st * 520 + DH, 65, HPC, 1, 1), 1.0)
                    else:
                        for g in range(4):
                            nc.tensor.matmul(
                                ps[:, 0:512],
                                ap3(xt8_all, (2 * g) * S + qc * 512 + st * 128,
                                    S, 2, 1, 128),
                                ap3(wv8_sb[g], 0, 512, 2, 1, 512),
                                start=(g == 0), stop=(g == 3),
                                perf_mode=DR)
                    nc.vector.tensor_copy(
                        ap3(vt8[stg >> 1], (stg & 1) * 640, 80, HPC, 1, DH),
                        ap3(ps, 0, DH, HPC, 1, DH))
                    nc.vector.memset(
                        ap3(vt8[stg >> 1], (stg & 1) * 640 + DH,
                            80, HPC, 1, 1), 1.0)

                def tail_hp(qc, hp, zt0, zt1):
                    lr = lrp.tile([1, 1024], f32, tag="lr")
                    nc.vector.tensor_copy(lr[:, 0:512], zt0[64:65, :])
                    nc.vector.tensor_copy(lr[:, 512:1024], zt1[64:65, :])
                    rr = lrp.tile([1, 1024], f32, tag="lr")
                    nc.vector.reciprocal_approx_fast(rr[:], lr[:])
                    rld = rldp.tile([1, 1024], f32, tag="rld")
                    nc.sync.dma_start(rld[:], rr[:])
                    lbs = lbsp.tile([DH, 1024], f32, tag="lbs")
                    nc.sync.dma_start(lbs[:], _bcast_ap(rld, 0, 0, DH, 1024))
                    zslice = zsum[:, qc * 512:(qc + 1) * 512]
                    for h, zt in ((0, zt0), (1, zt1)):
                        lb = lbs[:, h * 512:(h + 1) * 512]
                        if hp == 0 and h == 0:
                            nc.vector.tensor_tensor(
                                zslice, zt[0:DH, :], lb, op=ALU.mult)
                        else:
                            zn = znp.tile([DH, 512], f32, tag="zn")
                            nc.vector.tensor_tensor(
                                zn[:], zt[0:DH, :], lb, op=ALU.mult)
                            nc.vector.tensor_tensor(
                                zslice, zslice, zn[:], op=ALU.add)

                def attention0():
                    qc = 0
                    ets = {}

                    def pass1(hp):
                        lst = []
                        for kt in range(4):
                            j = kt
                            jw = j * 128
                            st2 = stp.tile([128, 1024], f32, tag="st2", name="st2")
                            nc.tensor.matmul(
                                st2[:, jw:512],
                                kt_all[0:64, hp * S + kt * 128:
                                       hp * S + (kt + 1) * 128],
                                qt_all[0:64, hp * S + jw: hp * S + 512],
                                start=True, stop=True, tile_position=(0, 0))
                            nc.tensor.matmul(
                                st2[:, 512 + jw:1024],
                                kt_all[64:128, hp * S + kt * 128:
                                       hp * S + (kt + 1) * 128],
                                qt_all[64:128, hp * S + jw: hp * S + 512],
                                start=True, stop=True, tile_position=(64, 0))
                            et = etp16.tile([128, 1024], f16, tag="et16",
                                            name="et16")
                            nc.scalar.activation(
                                ap3(et, jw, 512, 2, 1, 512 - jw),
                                ap3(st2, jw, 512, 2, 1, 512 - jw),
                                AF.Exp, scale=0.125)
                            for half in range(2):
                                blk = et[:, half * 512 + jw: half * 512 + jw + 128]
                                nc.gpsimd.affine_select(
                                    out=blk, in_=blk, compare_op=ALU.is_ge,
                                    fill=0.0, base=0, pattern=[[1, 128]],
                                    channel_multiplier=-1)
                            lst.append(et)
                        ets[hp] = lst

                    def pass2(hp):
                        zt0 = ztp.tile([65, 512], f32, tag="zt", name="zt0")
                        zt1 = ztp.tile([65, 512], f32, tag="zt", name="zt1")
                        for kt, et in enumerate(ets.pop(hp)):
                            jw = kt * 128
                            nc.tensor.matmul(
                                zt0[:, jw:512],
                                vt16[:, kt * 520 + (2 * hp) * 65:
                                     kt * 520 + (2 * hp) * 65 + 65],
                                et[:, jw:512],
                                start=(kt == 0), stop=(kt == 3))
                            nc.tensor.matmul(
                                zt1[:, jw:512],
                                vt16[:, kt * 520 + (2 * hp + 1) * 65:
                                     kt * 520 + (2 * hp + 1) * 65 + 65],
                                et[:, 512 + jw:1024],
                                start=(kt == 0), stop=(kt == 3))
                        tail_hp(qc, hp, zt0, zt1)

                    pass1(0)
                    for hp in range(4):
                        if hp < 3:
                            pass1(hp + 1)
                        pass2(hp)

                def attention(qc):
                    npairs = 2 * qc + 2
                    ets = {}

                    def pass1(hp):
                        lst = []
                        for pi in range(npairs):
                            et = etp8.tile([128, 2048], f8, tag="et8", name="et8")
                            for par in range(2):
                                kt = 2 * pi + par
                                j = kt - 4 * qc
                                jw = max(j, 0) * 128
                                st2 = stp.tile([128, 1024], f32, tag="st2",
                                               name="st2")
                                nc.tensor.matmul(
                                    st2[:, jw:512],
                                    kt_all[0:64, hp * S + kt * 128:
                                           hp * S + (kt + 1) * 128],
                                    qt_all[0:64, hp * S + qc * 512 + jw:
                                           hp * S + qc * 512 + 512],
                                    start=True, stop=True, tile_position=(0, 0))
                                nc.tensor.matmul(
                                    st2[:, 512 + jw:1024],
                                    kt_all[64:128, hp * S + kt * 128:
                                           hp * S + (kt + 1) * 128],
                                    qt_all[64:128, hp * S + qc * 512 + jw:
                                           hp * S + qc * 512 + 512],
                                    start=True, stop=True, tile_position=(64, 0))
                                nc.scalar.activation(
                                    ap3(et, par * 1024 + jw, 512, 2, 1, 512 - jw),
                                    ap3(st2, jw, 512, 2, 1, 512 - jw),
                                    AF.Exp, scale=0.125)
                                if j >= 0:
                                    for half in range(2):
                                        blk = et[:, par * 1024 + half * 512 + jw:
                                                 par * 1024 + half * 512 + jw + 128]
                                        nc.gpsimd.affine_select(
                                            out=blk, in_=blk, compare_op=ALU.is_ge,
                                            fill=0.0, base=0, pattern=[[1, 128]],
                                            channel_multiplier=-1)
                            lst.append(et)
                        ets[hp] = lst

                    def pass2(hp):
                        zt0 = ztp.tile([65, 512], f32, tag="zt", name="zt0")
                        zt1 = ztp.tile([65, 512], f32, tag="zt", name="zt1")
                        for pi, et in enumerate(ets.pop(hp)):
                            jE = 2 * pi - 4 * qc
                            if jE >= 0:
                                # diagonal pair: even-slot-only delta block as a
                                # plain fp8 matmul, DoubleRow on the shared window
                                d0, d1 = jE * 128, (jE + 1) * 128
                                for h, zt in ((0, zt0), (1, zt1)):
                                    hg = 2 * hp + h
                                    nc.tensor.matmul(
                                        zt[:, d0:d1],
                                        vt8[pi][:, hg * 80: hg * 80 + 65],
                                        et[:, h * 512 + d0: h * 512 + d1],
                                        start=False, stop=False)
                                    nc.tensor.matmul(
                                        zt[:, d1:512],
                                        ap3(vt8[pi], hg * 80, 640, 2, 1, 65),
                                        ap3(et, h * 512 + d1, 1024, 2, 1, 512 - d1),
                                        start=False, stop=(pi == npairs - 1),
                                        perf_mode=DR)
                            else:
                                for h, zt in ((0, zt0), (1, zt1)):
                                    hg = 2 * hp + h
                                    nc.tensor.matmul(
                                        zt[:, 0:512],
                                        ap3(vt8[pi], hg * 80, 640, 2, 1, 65),
                                        ap3(et, h * 512, 1024, 2, 1, 512),
                                        start=(pi == 0), stop=False,
                                        perf_mode=DR)
                        tail_hp(qc, hp, zt0, zt1)

                    pass1(0)
                    for hp in range(4):
                        if hp < 3:
                            pass1(hp + 1)
                        pass2(hp)

                def tail_proj(qc):
                    zsr = zrp.tile([128, 512], f16, tag="zsr")
                    nc.vector.tensor_copy(zsr[0:DH, :],
                                          zsum[:, qc * 512:(qc + 1) * 512])
                    nc.gpsimd.dma_start(zsr[DH:2 * DH, :],
                                        zsum[:, qc * 512:(qc + 1) * 512])
                    for qp in range(2):
                        for nn in range(2):
                            po = stp.tile([128, 1024], f32, tag="st2", name="po")
                            nc.tensor.matmul(
                                po[:, 0:512],
                                zsr[0:DH, (2 * qp) * 128:(2 * qp + 1) * 128],
                                wo_sb[0:DH, nn * 512:(nn + 1) * 512],
                                start=True, stop=True, tile_position=(0, 0))
                            nc.tensor.matmul(
                                po[:, 512:1024],
                                zsr[DH:128, (2 * qp + 1) * 128:(2 * qp + 2) * 128],
                                wo_sb[DH:128, nn * 512:(nn + 1) * 512],
                                start=True, stop=True, tile_position=(64, 0))
                            osb = osbp.tile([128, 1024], f32, tag="osb")
                            nc.vector.tensor_copy(osb[:], po[:])
                            r0 = qc * 512 + (2 * qp) * 128
                            nc.sync.dma_start(
                                out[r0:r0 + 128, nn * 512:(nn + 1) * 512],
                                osb[:, 0:512])
                            nc.sync.dma_start(
                                out[r0 + 128:r0 + 256, nn * 512:(nn + 1) * 512],
                                osb[:, 512:1024])

                for qc in range(NQ):
                    proj_x(qc)
                    for m in range(4):
                        proj_qk(qc, m)
                    for st in range(4):
                        proj_v(qc, st)
                    if qc == 0:
                        attention0()
                    else:
                        attention(qc)
                    if qc >= 1:
                        tail_proj(qc - 1)
                tail_proj(NQ - 1)
    nc.compile()
    return nc


def kernel(**inputs):
    import ml_dtypes
    f8 = ml_dtypes.float8_e4m3

    x = np.asarray(inputs["x"], dtype=np.float32)
    WQ = np.asarray(inputs["WQ"], dtype=np.float32)
    bQ = np.asarray(inputs["bQ"], dtype=np.float32)
    WK = np.asarray(inputs["WK"], dtype=np.float32)
    bK = np.asarray(inputs["bK"], dtype=np.float32)
    WV = np.asarray(inputs["WV"], dtype=np.float32)
    bV = np.asarray(inputs["bV"], dtype=np.float32)
    WO = np.asarray(inputs["WO"], dtype=np.float32)
    bO = np.asarray(inputs["bO"], dtype=np.float32)

    from concourse.bass_utils import run_bass_kernel_spmd

    if "nc" not in _prog:
        _prog["nc"] = _build()
    nc = _prog["nc"]

    def pair_layout(Wc):
        # [1024, 512] -> [512, 1024]: out[g*128+p, par*512+m] = Wc[g*256+par*128+p, m]
        return np.ascontiguousarray(
            Wc.reshape(4, 2, 128, GD).transpose(0, 2, 1, 3).reshape(512, 1024))

    in_maps = []
    for c in range(NCORES):
        b, g = c // 2, c % 2
        sl = slice(g * GD, (g + 1) * GD)
        xb = np.ascontiguousarray(x[b])
        in_maps.append({
            "x": xb.astype(np.float16),
            "wq8": pair_layout(WQ[:, sl]).astype(f8),
            "wk8": pair_layout(WK[:, sl]).astype(f8),
            "wv8": pair_layout(WV[:, sl]).astype(f8),
            "wv16": np.ascontiguousarray(WV[:, sl]).astype(np.float16),
            "bq": np.ascontiguousarray(bQ[sl]).reshape(1, GD).astype(np.float16),
            "bk": np.ascontiguousarray(bK[sl]).reshape(1, GD).astype(np.float16),
            "wo": WO.astype(np.float16),
        })
    _prog["in_maps"] = in_maps
    res = run_bass_kernel_spmd(nc, in_maps, core_ids=list(range(NCORES)))
    parts = [r["out"] for r in res.results]

    extra = bV.reshape(H, DH).sum(0) @ WO + np.float32(H) * bO
    out = np.empty((B, S, D), dtype=np.float32)
    for b in range(B):
        out[b] = parts[2 * b] + parts[2 * b + 1] + extra
    return out


# revision 15
# speedup vs baseline: 1.3097x; 1.0636x over previous
"""Causal self-attention kernel for 8 Trainium2 NeuronCores.

Sharding: core c -> (batch b = c//2, head-group g = c%2). Each core computes
the attention output contribution of 8 heads for one batch element:
    P_c = (sum_{h in group} softmax(Q_h K_h^T / 8 + causal) V_h) @ WO
Host epilogue: out[b] = P_{2b} + P_{2b+1} + (sum_h bV_h) @ WO + 16*bO
(the V-bias commutes through softmax normalization: softmax rows sum to 1).

Precision/speed split (PE row counts at ~1.5 G rows/s are the limiter):
  - Q/K/V projections run as fp8-E4M3 DoubleRow matmuls (K=256 per pass:
    d-tile pairs via a [128, 2, N] AP over the existing xt layout), halving
    projection PE time. Chunk 0's V is also computed in fp16 (kept in vt16)
    because output rows with tiny softmax support (q < 512) see V error
    directly; all other chunks' softmax averages 512+ values so fp8 noise
    washes out.
  - Scores stay fp16 (K=64 contraction gets no DoubleRow benefit); two heads
    per PE pass via tile_position row-tiling. Q/K fp8 projection error only
    perturbs softmax weights (benign at any support size).
  - A@V for q-chunks >= 1 runs fp8 DoubleRow over k-tile PAIRS: ET pair
    tiles [128, 2, 2head*512] written by exp directly in fp8, V pair tiles
    [128, 2, 8head*65] with a ones column accumulating the softmax
    denominator in row 64. Chunk 0 uses the fp16 per-k-tile path.
  - Diagonal k-tiles shrink the ST matmul, exp, and ZT to the unmasked
    q-window; fully masked regions are never computed (no memsets except the
    odd-slot delta of diagonal fp8 pairs). The in-block triangle is masked
    by gpsimd affine_select on ET after exp.
  - Per-hp tail: 1/l via DVE reciprocal_approx_fast (no ScalarE Ln/Exp table
    swaps), broadcast by a DRAM-bounce DMA, normalize straight out of zt
    PSUM into the zsum accumulator, so almost nothing serializes at the end.
"""
import numpy as np

B, S, D, H, DH = 4, 2048, 1024, 16, 64
HPC = 8            # heads per core
GD = HPC * DH      # 512 = group width
NCORES = 8
NQ = S // 512      # 4 q/s chunks of 512
NKT = S // 128     # 16 k-tiles
NDT = D // 128     # 8 d-tiles

_prog = {}


def ap3(tile_t, offset, d1, n1, d2, n2):
    """AP view [128p, n1, n2] over a tile's free dim: col = offset + i*d1 + j*d2."""
    import concourse.bass as bass
    ap = tile_t[:]
    return bass.AP(ap.tensor, ap.offset + offset,
                   [ap.ap[0], [d1, n1], [d2, n2]])


def bass_ap_3d(tile_t, offset, stride, n, inner):
    return ap3(tile_t, offset, stride, n, 1, inner)


def _bcast_ap(tile_t, row, col, nparts, width):
    """Partition-step-0 AP reading (row, col:col+width) replicated nparts times."""
    import concourse.bass as bass
    ap = tile_t[:]
    pstep = ap.ap[0][0]
    return bass.AP(ap.tensor, ap.offset + row * pstep + col,
                   [[0, nparts], [1, width]])


def _build():
    import concourse.bacc as bacc
    import concourse.tile as tile
    from concourse import mybir
    import concourse.bass as bass

    f32 = mybir.dt.float32
    f16 = mybir.dt.float16
    f8 = mybir.dt.float8e4
    AF = mybir.ActivationFunctionType
    ALU = mybir.AluOpType
    DR = mybir.MatmulPerfMode.DoubleRow

    nc = bacc.Bacc(None, target_bir_lowering=False, debug=False)
    x = nc.dram_tensor("x", [S, D], f16, kind="ExternalInput")
    # wq8/wk8/wv8: [4*128, 2*512]: row = g*128 + p, col = par*512 + m,
    # value = W[g*256 + par*128 + p, m]  (d-pair DoubleRow layout)
    wq8 = nc.dram_tensor("wq8", [512, 1024], f8, kind="ExternalInput")
    wk8 = nc.dram_tensor("wk8", [512, 1024], f8, kind="ExternalInput")
    wv8 = nc.dram_tensor("wv8", [512, 1024], f8, kind="ExternalInput")
    wv16 = nc.dram_tensor("wv16", [D, GD], f16, kind="ExternalInput")
    bq = nc.dram_tensor("bq", [1, GD], f16, kind="ExternalInput")
    bk = nc.dram_tensor("bk", [1, GD], f16, kind="ExternalInput")
    wo = nc.dram_tensor("wo", [DH, D], f16, kind="ExternalInput")
    out = nc.dram_tensor("out", [S, D], f32, kind="ExternalOutput")

    with tile.TileContext(nc) as tc:
        with tc.tile_pool(name="const", bufs=1) as constp, \
             tc.tile_pool(name="big", bufs=1) as bigp:
            idt16 = constp.tile([128, 128], f16, tag="idt16")
            from concourse.masks import make_identity
            make_identity(nc, idt16[:])
            bq_t = constp.tile([128, 4], f32, tag="bq_t")
            bk_t = constp.tile([128, 4], f32, tag="bk_t")
            nc.gpsimd.dma_start(bq_t[:], bass.AP(bq, 0, [[1, 128], [128, 4]]))
            nc.gpsimd.dma_start(bk_t[:], bass.AP(bk, 0, [[1, 128], [128, 4]]))
            wo_sb = constp.tile([128, D], f16, tag="wo_sb")
            nc.gpsimd.dma_start(wo_sb[0:DH, :], wo[:])
            nc.gpsimd.dma_start(wo_sb[DH:2 * DH, :], wo[:])

            # persistent per-core tensors
            xt8_all = bigp.tile([128, NDT * S], f8, tag="xt8")    # d-tile j at j*S
            xt16_c0 = bigp.tile([128, NDT * 512], f16, tag="xt16")  # d-tile j at j*512
            qt_all = bigp.tile([128, 4 * S], f16, tag="qt")       # m-tile m at m*S
            kt_all = bigp.tile([128, 4 * S], f16, tag="kt")
            # vt8: k-tile pair p: [par(2) x head(8) x 65] at col par*520+h*65+c
            vt8 = [bigp.tile([128, 1280], f8, tag=f"vt8_{p}", name=f"vt8_{p}")
                   for p in range(NKT // 2)]
            vt16 = bigp.tile([128, 4 * 520], f16, tag="vt16")     # chunk-0 k-tiles
            zsum = bigp.tile([DH, S], f32, tag="zsum")

            with tc.tile_pool(name="wts", bufs=1) as wtp, \
                 tc.tile_pool(name="xs", bufs=5) as xsp, \
                 tc.tile_pool(name="et16", bufs=6) as etp16, \
                 tc.tile_pool(name="et8", bufs=16) as etp8, \
                 tc.tile_pool(name="rld", bufs=2, space="DRAM") as rldp, \
                 tc.tile_pool(name="lr", bufs=2) as lrp, \
                 tc.tile_pool(name="lbs", bufs=2) as lbsp, \
                 tc.tile_pool(name="zn", bufs=2) as znp, \
                 tc.tile_pool(name="zr", bufs=2) as zrp, \
                 tc.tile_pool(name="osb", bufs=3) as osbp, \
                 tc.tile_pool(name="stq", bufs=2, space="PSUM") as stq, \
                 tc.tile_pool(name="prp", bufs=2, space="PSUM") as prp, \
                 tc.tile_pool(name="ztp", bufs=2, space="PSUM") as ztp:
                wq8_sb = [wtp.tile([128, 1024], f8, tag=f"wq{k}", name=f"wq{k}")
                          for k in range(4)]
                wk8_sb = [wtp.tile([128, 1024], f8, tag=f"wk{k}", name=f"wk{k}")
                          for k in range(4)]
                wv8_sb = [wtp.tile([128, 1024], f8, tag=f"wv{k}", name=f"wv{k}")
                          for k in range(4)]
                wv16_sb = [wtp.tile([128, GD], f16, tag=f"wv16_{k}",
                                    name=f"wv16_{k}") for k in range(NDT)]
                for k in range(4):
                    nc.scalar.dma_start(wq8_sb[k][:], wq8[k * 128:(k + 1) * 128, :])
                    nc.scalar.dma_start(wk8_sb[k][:], wk8[k * 128:(k + 1) * 128, :])
                    nc.gpsimd.dma_start(wv8_sb[k][:], wv8[k * 128:(k + 1) * 128, :])
                for k in range(NDT):
                    nc.gpsimd.dma_start(wv16_sb[k][:], wv16[k * 128:(k + 1) * 128, :])

                def proj_x(qc):
                    xss = []
                    for st4 in range(4):
                        srow = qc * 512 + st4 * 128
                        xs = xsp.tile([128, D], f16, tag="xs", name="xs")
                        nc.sync.dma_start(xs[:], x[srow:srow + 128, :])
                        xss.append(xs)
                    # transpose x into xT via the PE, 8 per 1-bank PSUM tile;
                    # eviction casts to fp8 (and keeps fp16 for chunk 0)
                    for jj in range(4):
                        pt = prp.tile([128, 1024], f16, tag="pr1", name="pt")
                        for j2 in range(2):
                            j = jj * 2 + j2
                            for st4 in range(4):
                                nc.tensor.transpose(
                                    pt[:, j2 * 512 + st4 * 128:
                                       j2 * 512 + (st4 + 1) * 128],
                                    xss[st4][:, j * 128:(j + 1) * 128], idt16[:])
                        dst = bass_ap_3d(xt8_all, (jj * 2) * S + qc * 512, S, 2, 512)
                        nc.vector.tensor_copy(dst, bass_ap_3d(pt, 0, 512, 2, 512))
                        if qc == 0:
                            dst16 = bass_ap_3d(xt16_c0, (jj * 2) * 512, 512, 2, 512)
                            nc.vector.tensor_copy(dst16,
                                                  bass_ap_3d(pt, 0, 512, 2, 512))
                def proj_qk(qc, m):
                    # Q/K m-tile: fp8 DoubleRow over d-pairs, 1-bank accumulators
                    for (w_sb, b_t, dest) in ((wq8_sb, bq_t, qt_all),
                                              (wk8_sb, bk_t, kt_all)):
                        ps = prp.tile([128, 512], f32, tag="pr1", name="ps")
                        for g in range(4):
                            nc.tensor.matmul(
                                ps[:],
                                ap3(w_sb[g], m * 128, 512, 2, 1, 128),
                                ap3(xt8_all, (2 * g) * S + qc * 512,
                                    S, 2, 1, 512),
                                start=(g == 0), stop=(g == 3),
                                perf_mode=DR)
                        nc.vector.tensor_scalar_add(
                            dest[:, m * S + qc * 512: m * S + (qc + 1) * 512],
                            ps[:], b_t[:, m:m + 1])

                def proj_v(qc, st):
                    stg = qc * 4 + st
                    ps = prp.tile([128, 512], f32, tag="pr1", name="ps")
                    if qc == 0:
                        for k in range(NDT):
                            nc.tensor.matmul(
                                ps[:],
                                xt16_c0[:, k * 512 + st * 128:
                                        k * 512 + (st + 1) * 128],
                                wv16_sb[k][:],
                                start=(k == 0), stop=(k == NDT - 1))
                        src = ap3(ps, 0, DH, HPC, 1, DH)
                        nc.vector.tensor_copy(
                            ap3(vt16, st * 520, 65, HPC, 1, DH), src)
                        nc.vector.memset(
                            ap3(vt16, st * 520 + DH, 65, HPC, 1, 1), 1.0)
                    else:
                        for g in range(4):
                            nc.tensor.matmul(
                                ps[:],
                                ap3(xt8_all, (2 * g) * S + qc * 512 + st * 128,
                                    S, 2, 1, 128),
                                ap3(wv8_sb[g], 0, 512, 2, 1, 512),
                                start=(g == 0), stop=(g == 3),
                                perf_mode=DR)
                    nc.vector.tensor_copy(
                        ap3(vt8[stg >> 1], (stg & 1) * 640, 80, HPC, 1, DH),
                        ap3(ps, 0, DH, HPC, 1, DH))
                    nc.vector.memset(
                        ap3(vt8[stg >> 1], (stg & 1) * 640 + DH,
                            80, HPC, 1, 1), 1.0)

                def tail_hp(qc, hp, zt0, zt1):
                    lr = lrp.tile([1, 1024], f32, tag="lr")
                    nc.vector.tensor_copy(lr[:, 0:512], zt0[64:65, :])
                    nc.vector.tensor_copy(lr[:, 512:1024], zt1[64:65, :])
                    rr = lrp.tile([1, 1024], f32, tag="lr")
                    nc.vector.reciprocal_approx_fast(rr[:], lr[:])
                    rld = rldp.tile([1, 1024], f32, tag="rld")
                    nc.sync.dma_start(rld[:], rr[:])
                    lbs = lbsp.tile([DH, 1024], f32, tag="lbs")
                    nc.sync.dma_start(lbs[:], _bcast_ap(rld, 0, 0, DH, 1024))
                    zslice = zsum[:, qc * 512:(qc + 1) * 512]
                    for h, zt in ((0, zt0), (1, zt1)):
                        lb = lbs[:, h * 512:(h + 1) * 512]
                        if hp == 0 and h == 0:
                            nc.vector.tensor_tensor(
                                zslice, zt[0:DH, :], lb, op=ALU.mult)
                        else:
                            zn = znp.tile([DH, 512], f32, tag="zn")
                            nc.vector.tensor_tensor(
                                zn[:], zt[0:DH, :], lb, op=ALU.mult)
                            nc.vector.tensor_tensor(
                                zslice, zslice, zn[:], op=ALU.add)

                def attention0():
                    qc = 0
                    ets = {}

                    def pass1(hp):
                        lst = []
                        for kt in range(4):
                            j = kt
                            jw = j * 128
                            st2 = stq.tile([128, 1024], f32, tag="st2", name="st2")
                            nc.tensor.matmul(
                                st2[:, jw:512],
                                kt_all[0:64, hp * S + kt * 128:
                                       hp * S + (kt + 1) * 128],
                                qt_all[0:64, hp * S + jw: hp * S + 512],
                                start=True, stop=True, tile_position=(0, 0))
                            nc.tensor.matmul(
                                st2[:, 512 + jw:1024],
                                kt_all[64:128, hp * S + kt * 128:
                                       hp * S + (kt + 1) * 128],
                                qt_all[64:128, hp * S + jw: hp * S + 512],
                                start=True, stop=True, tile_position=(64, 0))
                            et = etp16.tile([128, 1024], f16, tag="et16",
                                            name="et16")
                            nc.scalar.activation(
                                ap3(et, jw, 512, 2, 1, 512 - jw),
                                ap3(st2, jw, 512, 2, 1, 512 - jw),
                                AF.Exp, scale=0.125)
                            for half in range(2):
                                blk = et[:, half * 512 + jw: half * 512 + jw + 128]
                                nc.gpsimd.affine_select(
                                    out=blk, in_=blk, compare_op=ALU.is_ge,
                                    fill=0.0, base=0, pattern=[[1, 128]],
                                    channel_multiplier=-1)
                            lst.append(et)
                        ets[hp] = lst

                    def pass2(hp):
                        zt0 = ztp.tile([65, 512], f32, tag="zt", name="zt0")
                        zt1 = ztp.tile([65, 512], f32, tag="zt", name="zt1")
                        for kt, et in enumerate(ets.pop(hp)):
                            jw = kt * 128
                            nc.tensor.matmul(
                                zt0[:, jw:512],
                                vt16[:, kt * 520 + (2 * hp) * 65:
                                     kt * 520 + (2 * hp) * 65 + 65],
                                et[:, jw:512],
                                start=(kt == 0), stop=(kt == 3))
                            nc.tensor.matmul(
                                zt1[:, jw:512],
                                vt16[:, kt * 520 + (2 * hp + 1) * 65:
                                     kt * 520 + (2 * hp + 1) * 65 + 65],
                                et[:, 512 + jw:1024],
                                start=(kt == 0), stop=(kt == 3))
                        tail_hp(qc, hp, zt0, zt1)

                    pass1(0)
                    for hp in range(4):
                        if hp < 3:
                            pass1(hp + 1)
                        pass2(hp)

                def attention(qc):
                    npairs = 2 * qc + 2
                    ets = {}

                    def pass1(hp):
                        lst = []
                        for pi in range(npairs):
                            et = etp8.tile([128, 2048], f8, tag="et8", name="et8")
                            for par in range(2):
                                kt = 2 * pi + par
                                j = kt - 4 * qc
                                jw = max(j, 0) * 128
                                st2 = stq.tile([128, 1024], f32, tag="st2",
                                               name="st2")
                                nc.tensor.matmul(
                                    st2[:, jw:512],
                                    kt_all[0:64, hp * S + kt * 128:
                                           hp * S + (kt + 1) * 128],
                                    qt_all[0:64, hp * S + qc * 512 + jw:
                                           hp * S + qc * 512 + 512],
                                    start=True, stop=True, tile_position=(0, 0))
                                nc.tensor.matmul(
                                    st2[:, 512 + jw:1024],
                                    kt_all[64:128, hp * S + kt * 128:
                                           hp * S + (kt + 1) * 128],
                                    qt_all[64:128, hp * S + qc * 512 + jw:
                                           hp * S + qc * 512 + 512],
                                    start=True, stop=True, tile_position=(64, 0))
                                nc.scalar.activation(
                                    ap3(et, par * 1024 + jw, 512, 2, 1, 512 - jw),
                                    ap3(st2, jw, 512, 2, 1, 512 - jw),
                                    AF.Exp, scale=0.125)
                                if j >= 0:
                                    for half in range(2):
                                        blk = et[:, par * 1024 + half * 512 + jw:
                                                 par * 1024 + half * 512 + jw + 128]
                                        nc.gpsimd.affine_select(
                                            out=blk, in_=blk, compare_op=ALU.is_ge,
                                            fill=0.0, base=0, pattern=[[1, 128]],
                                            channel_multiplier=-1)
                            lst.append(et)
                        ets[hp] = lst

                    def pass2(hp):
                        zt0 = ztp.tile([65, 512], f32, tag="zt", name="zt0")
                        zt1 = ztp.tile([65, 512], f32, tag="zt", name="zt1")
                        for pi, et in enumerate(ets.pop(hp)):
                            jE = 2 * pi - 4 * qc
                            if jE >= 0:
                                # diagonal pair: even-slot-only delta block as a
                                # plain fp8 matmul, DoubleRow on the shared window
                                d0, d1 = jE * 128, (jE + 1) * 128
                                for h, zt in ((0, zt0), (1, zt1)):
                                    hg = 2 * hp + h
                                    nc.tensor.matmul(
                                        zt[:, d0:d1],
                                        vt8[pi][:, hg * 80: hg * 80 + 65],
                                        et[:, h * 512 + d0: h * 512 + d1],
                                        start=False, stop=False)
                                    nc.tensor.matmul(
                                        zt[:, d1:512],
                                        ap3(vt8[pi], hg * 80, 640, 2, 1, 65),
                                        ap3(et, h * 512 + d1, 1024, 2, 1, 512 - d1),
                                        start=False, stop=(pi == npairs - 1),
                                        perf_mode=DR)
                            else:
                                for h, zt in ((0, zt0), (1, zt1)):
                                    hg = 2 * hp + h
                                    nc.tensor.matmul(
                                        zt[:, 0:512],
                                        ap3(vt8[pi], hg * 80, 640, 2, 1, 65),
                                        ap3(et, h * 512, 1024, 2, 1, 512),
                                        start=(pi == 0), stop=False,
                                        perf_mode=DR)
                        tail_hp(qc, hp, zt0, zt1)

                    pass1(0)
                    for hp in range(4):
                        if hp < 3:
                            pass1(hp + 1)
                        pass2(hp)

                def tail_proj(qc):
                    zsr = zrp.tile([128, 512], f16, tag="zsr")
                    nc.vector.tensor_copy(zsr[0:DH, :],
                                          zsum[:, qc * 512:(qc + 1) * 512])
                    nc.gpsimd.dma_start(zsr[DH:2 * DH, :],
                                        zsum[:, qc * 512:(qc + 1) * 512])
                    for qp in range(2):
                        for nn in range(2):
                            po0 = prp.tile([128, 512], f32, tag="pr1", name="po0")
                            po1 = prp.tile([128, 512], f32, tag="pr1", name="po1")
                            nc.tensor.matmul(
                                po0[:],
                                zsr[0:DH, (2 * qp) * 128:(2 * qp + 1) * 128],
                                wo_sb[0:DH, nn * 512:(nn + 1) * 512],
                                start=True, stop=True, tile_position=(0, 0))
                            nc.tensor.matmul(
                                po1[:],
                                zsr[DH:128, (2 * qp + 1) * 128:(2 * qp + 2) * 128],
                                wo_sb[DH:128, nn * 512:(nn + 1) * 512],
                                start=True, stop=True, tile_position=(64, 0))
                            osb = osbp.tile([128, 1024], f32, tag="osb")
                            nc.vector.tensor_copy(osb[:, 0:512], po0[:])
                            nc.vector.tensor_copy(osb[:, 512:1024], po1[:])
                            r0 = qc * 512 + (2 * qp) * 128
                            nc.sync.dma_start(
                                out[r0:r0 + 128, nn * 512:(nn + 1) * 512],
                                osb[:, 0:512])
                            nc.sync.dma_start(
                                out[r0 + 128:r0 + 256, nn * 512:(nn + 1) * 512],
                                osb[:, 512:1024])

                for qc in range(NQ):
                    proj_x(qc)
                    for m in range(4):
                        proj_qk(qc, m)
                    for st in range(4):
                        proj_v(qc, st)
                    if qc == 0:
                        attention0()
                    else:
                        attention(qc)
                    if qc >= 1:
                        tail_proj(qc - 1)
                tail_proj(NQ - 1)
    nc.compile()
    return nc


def kernel(**inputs):
    import ml_dtypes
    f8 = ml_dtypes.float8_e4m3

    x = np.asarray(inputs["x"], dtype=np.float32)
    WQ = np.asarray(inputs["WQ"], dtype=np.float32)
    bQ = np.asarray(inputs["bQ"], dtype=np.float32)
    WK = np.asarray(inputs["WK"], dtype=np.float32)
    bK = np.asarray(inputs["bK"], dtype=np.float32)
    WV = np.asarray(inputs["WV"], dtype=np.float32)
    bV = np.asarray(inputs["bV"], dtype=np.float32)
    WO = np.asarray(inputs["WO"], dtype=np.float32)
    bO = np.asarray(inputs["bO"], dtype=np.float32)

    from concourse.bass_utils import run_bass_kernel_spmd

    if "nc" not in _prog:
        _prog["nc"] = _build()
    nc = _prog["nc"]

    def pair_layout(Wc):
        # [1024, 512] -> [512, 1024]: out[g*128+p, par*512+m] = Wc[g*256+par*128+p, m]
        return np.ascontiguousarray(
            Wc.reshape(4, 2, 128, GD).transpose(0, 2, 1, 3).reshape(512, 1024))

    in_maps = []
    for c in range(NCORES):
        b, g = c // 2, c % 2
        sl = slice(g * GD, (g + 1) * GD)
        xb = np.ascontiguousarray(x[b])
        in_maps.append({
            "x": xb.astype(np.float16),
            "wq8": pair_layout(WQ[:, sl]).astype(f8),
            "wk8": pair_layout(WK[:, sl]).astype(f8),
            "wv8": pair_layout(WV[:, sl]).astype(f8),
            "wv16": np.ascontiguousarray(WV[:, sl]).astype(np.float16),
            "bq": np.ascontiguousarray(bQ[sl]).reshape(1, GD).astype(np.float16),
            "bk": np.ascontiguousarray(bK[sl]).reshape(1, GD).astype(np.float16),
            "wo": WO.astype(np.float16),
        })
    _prog["in_maps"] = in_maps
    res = run_bass_kernel_spmd(nc, in_maps, core_ids=list(range(NCORES)))
    parts = [r["out"] for r in res.results]

    extra = bV.reshape(H, DH).sum(0) @ WO + np.float32(H) * bO
    out = np.empty((B, S, D), dtype=np.float32)
    for b in range(B):
        out[b] = parts[2 * b] + parts[2 * b + 1] + extra
    return out


# revision 16
# speedup vs baseline: 1.3297x; 1.0153x over previous
"""Causal self-attention kernel for 8 Trainium2 NeuronCores.

Sharding: core c -> (batch b = c//2, head-group g = c%2). Each core computes
the attention output contribution of 8 heads for one batch element:
    P_c = (sum_{h in group} softmax(Q_h K_h^T / 8 + causal) V_h) @ WO
Host epilogue: out[b] = P_{2b} + P_{2b+1} + (sum_h bV_h) @ WO + 16*bO
(the V-bias commutes through softmax normalization: softmax rows sum to 1).

Precision/speed split (PE row counts at ~1.5 G rows/s are the limiter):
  - Q/K/V projections run as fp8-E4M3 DoubleRow matmuls (K=256 per pass:
    d-tile pairs via a [128, 2, N] AP over the existing xt layout), halving
    projection PE time. Chunk 0's V is also computed in fp16 (kept in vt16)
    because output rows with tiny softmax support (q < 512) see V error
    directly; all other chunks' softmax averages 512+ values so fp8 noise
    washes out.
  - Scores stay fp16 (K=64 contraction gets no DoubleRow benefit); two heads
    per PE pass via tile_position row-tiling. Q/K fp8 projection error only
    perturbs softmax weights (benign at any support size).
  - A@V for q-chunks >= 1 runs fp8 DoubleRow over k-tile PAIRS: ET pair
    tiles [128, 2, 2head*512] written by exp directly in fp8, V pair tiles
    [128, 2, 8head*65] with a ones column accumulating the softmax
    denominator in row 64. Chunk 0 uses the fp16 per-k-tile path.
  - Diagonal k-tiles shrink the ST matmul, exp, and ZT to the unmasked
    q-window; fully masked regions are never computed (no memsets except the
    odd-slot delta of diagonal fp8 pairs). The in-block triangle is masked
    by gpsimd affine_select on ET after exp.
  - Per-hp tail: 1/l via DVE reciprocal_approx_fast (no ScalarE Ln/Exp table
    swaps), broadcast by a DRAM-bounce DMA, normalize straight out of zt
    PSUM into the zsum accumulator, so almost nothing serializes at the end.
"""
import numpy as np

B, S, D, H, DH = 4, 2048, 1024, 16, 64
HPC = 8            # heads per core
GD = HPC * DH      # 512 = group width
NCORES = 8
NQ = S // 512      # 4 q/s chunks of 512
NKT = S // 128     # 16 k-tiles
NDT = D // 128     # 8 d-tiles

_prog = {}


def ap3(tile_t, offset, d1, n1, d2, n2):
    """AP view [128p, n1, n2] over a tile's free dim: col = offset + i*d1 + j*d2."""
    import concourse.bass as bass
    ap = tile_t[:]
    return bass.AP(ap.tensor, ap.offset + offset,
                   [ap.ap[0], [d1, n1], [d2, n2]])


def bass_ap_3d(tile_t, offset, stride, n, inner):
    return ap3(tile_t, offset, stride, n, 1, inner)


def _bcast_ap(tile_t, row, col, nparts, width):
    """Partition-step-0 AP reading (row, col:col+width) replicated nparts times."""
    import concourse.bass as bass
    ap = tile_t[:]
    pstep = ap.ap[0][0]
    return bass.AP(ap.tensor, ap.offset + row * pstep + col,
                   [[0, nparts], [1, width]])


def _build():
    import concourse.bacc as bacc
    import concourse.tile as tile
    from concourse import mybir
    import concourse.bass as bass

    f32 = mybir.dt.float32
    f16 = mybir.dt.float16
    f8 = mybir.dt.float8e4
    AF = mybir.ActivationFunctionType
    ALU = mybir.AluOpType
    DR = mybir.MatmulPerfMode.DoubleRow

    nc = bacc.Bacc(None, target_bir_lowering=False, debug=False)
    x = nc.dram_tensor("x", [S, D], f16, kind="ExternalInput")
    # wq8/wk8/wv8: [4*128, 2*512]: row = g*128 + p, col = par*512 + m,
    # value = W[g*256 + par*128 + p, m]  (d-pair DoubleRow layout)
    wq8 = nc.dram_tensor("wq8", [512, 1024], f8, kind="ExternalInput")
    wk8 = nc.dram_tensor("wk8", [512, 1024], f8, kind="ExternalInput")
    wv8 = nc.dram_tensor("wv8", [512, 1024], f8, kind="ExternalInput")
    wv16 = nc.dram_tensor("wv16", [D, GD], f16, kind="ExternalInput")
    bq = nc.dram_tensor("bq", [1, GD], f16, kind="ExternalInput")
    bk = nc.dram_tensor("bk", [1, GD], f16, kind="ExternalInput")
    wo = nc.dram_tensor("wo", [DH, D], f16, kind="ExternalInput")
    out = nc.dram_tensor("out", [S, D], f32, kind="ExternalOutput")

    with tile.TileContext(nc) as tc:
        with tc.tile_pool(name="const", bufs=1) as constp, \
             tc.tile_pool(name="big", bufs=1) as bigp:
            idt16 = constp.tile([128, 128], f16, tag="idt16")
            from concourse.masks import make_identity
            make_identity(nc, idt16[:])
            bq_t = constp.tile([128, 4], f32, tag="bq_t")
            bk_t = constp.tile([128, 4], f32, tag="bk_t")
            nc.gpsimd.dma_start(bq_t[:], bass.AP(bq, 0, [[1, 128], [128, 4]]))
            nc.gpsimd.dma_start(bk_t[:], bass.AP(bk, 0, [[1, 128], [128, 4]]))
            wo_sb = constp.tile([128, D], f16, tag="wo_sb")
            nc.gpsimd.dma_start(wo_sb[0:DH, :], wo[:])
            nc.gpsimd.dma_start(wo_sb[DH:2 * DH, :], wo[:])

            # persistent per-core tensors
            xt8_all = bigp.tile([128, NDT * S], f8, tag="xt8")    # d-tile j at j*S
            xt16_c0 = bigp.tile([128, NDT * 512], f16, tag="xt16")  # d-tile j at j*512
            qt_all = bigp.tile([128, 4 * S], f16, tag="qt")       # m-tile m at m*S
            kt_all = bigp.tile([128, 4 * S], f16, tag="kt")
            # vt8: k-tile pair p: [par(2) x head(8) x 65] at col par*520+h*65+c
            vt8 = [bigp.tile([128, 1280], f8, tag=f"vt8_{p}", name=f"vt8_{p}")
                   for p in range(NKT // 2)]
            vt16 = bigp.tile([128, 4 * 520], f16, tag="vt16")     # chunk-0 k-tiles
            zsum = bigp.tile([DH, S], f16, tag="zsum")

            with tc.tile_pool(name="wts", bufs=1) as wtp, \
                 tc.tile_pool(name="xs", bufs=5) as xsp, \
                 tc.tile_pool(name="et16", bufs=6) as etp16, \
                 tc.tile_pool(name="et8", bufs=16) as etp8, \
                 tc.tile_pool(name="rld", bufs=2, space="DRAM") as rldp, \
                 tc.tile_pool(name="lr", bufs=2) as lrp, \
                 tc.tile_pool(name="lbs", bufs=2) as lbsp, \
                 tc.tile_pool(name="zn", bufs=2) as znp, \
                 tc.tile_pool(name="zr", bufs=2) as zrp, \
                 tc.tile_pool(name="osb", bufs=3) as osbp, \
                 tc.tile_pool(name="stq", bufs=2, space="PSUM") as stq, \
                 tc.tile_pool(name="prp", bufs=2, space="PSUM") as prp, \
                 tc.tile_pool(name="ztp", bufs=2, space="PSUM") as ztp:
                wq8_sb = [wtp.tile([128, 1024], f8, tag=f"wq{k}", name=f"wq{k}")
                          for k in range(4)]
                wk8_sb = [wtp.tile([128, 1024], f8, tag=f"wk{k}", name=f"wk{k}")
                          for k in range(4)]
                wv8_sb = [wtp.tile([128, 1024], f8, tag=f"wv{k}", name=f"wv{k}")
                          for k in range(4)]
                wv16_sb = [wtp.tile([128, GD], f16, tag=f"wv16_{k}",
                                    name=f"wv16_{k}") for k in range(NDT)]
                for k in range(4):
                    nc.scalar.dma_start(wq8_sb[k][:], wq8[k * 128:(k + 1) * 128, :])
                    nc.scalar.dma_start(wk8_sb[k][:], wk8[k * 128:(k + 1) * 128, :])
                    nc.scalar.dma_start(wv8_sb[k][:], wv8[k * 128:(k + 1) * 128, :])
                for k in range(NDT):
                    nc.scalar.dma_start(wv16_sb[k][:], wv16[k * 128:(k + 1) * 128, :])

                def proj_x(qc):
                    xss = []
                    for st4 in range(4):
                        srow = qc * 512 + st4 * 128
                        xs = xsp.tile([128, D], f16, tag="xs", name="xs")
                        nc.sync.dma_start(xs[:], x[srow:srow + 128, :])
                        xss.append(xs)
                    # transpose x into xT via the PE, 8 per 1-bank PSUM tile;
                    # eviction casts to fp8 (and keeps fp16 for chunk 0)
                    for jj in range(4):
                        pt = prp.tile([128, 1024], f16, tag="pr1", name="pt")
                        for j2 in range(2):
                            j = jj * 2 + j2
                            for st4 in range(4):
                                nc.tensor.transpose(
                                    pt[:, j2 * 512 + st4 * 128:
                                       j2 * 512 + (st4 + 1) * 128],
                                    xss[st4][:, j * 128:(j + 1) * 128], idt16[:])
                        dst = bass_ap_3d(xt8_all, (jj * 2) * S + qc * 512, S, 2, 512)
                        nc.vector.tensor_copy(dst, bass_ap_3d(pt, 0, 512, 2, 512))
                        if qc == 0:
                            dst16 = bass_ap_3d(xt16_c0, (jj * 2) * 512, 512, 2, 512)
                            nc.vector.tensor_copy(dst16,
                                                  bass_ap_3d(pt, 0, 512, 2, 512))
                def proj_qk(qc, m):
                    # Q/K m-tile: fp8 DoubleRow over d-pairs, 1-bank accumulators
                    for (w_sb, b_t, dest) in ((wq8_sb, bq_t, qt_all),
                                              (wk8_sb, bk_t, kt_all)):
                        ps = prp.tile([128, 512], f32, tag="pr1", name="ps")
                        for g in range(4):
                            nc.tensor.matmul(
                                ps[:],
                                ap3(w_sb[g], m * 128, 512, 2, 1, 128),
                                ap3(xt8_all, (2 * g) * S + qc * 512,
                                    S, 2, 1, 512),
                                start=(g == 0), stop=(g == 3),
                                perf_mode=DR)
                        nc.vector.tensor_scalar_add(
                            dest[:, m * S + qc * 512: m * S + (qc + 1) * 512],
                            ps[:], b_t[:, m:m + 1])

                def proj_v(qc, st):
                    stg = qc * 4 + st
                    ps = prp.tile([128, 512], f32, tag="pr1", name="ps")
                    if qc == 0:
                        for k in range(NDT):
                            nc.tensor.matmul(
                                ps[:],
                                xt16_c0[:, k * 512 + st * 128:
                                        k * 512 + (st + 1) * 128],
                                wv16_sb[k][:],
                                start=(k == 0), stop=(k == NDT - 1))
                        src = ap3(ps, 0, DH, HPC, 1, DH)
                        nc.vector.tensor_copy(
                            ap3(vt16, st * 520, 65, HPC, 1, DH), src)
                        nc.vector.memset(
                            ap3(vt16, st * 520 + DH, 65, HPC, 1, 1), 1.0)
                    else:
                        for g in range(4):
                            nc.tensor.matmul(
                                ps[:],
                                ap3(xt8_all, (2 * g) * S + qc * 512 + st * 128,
                                    S, 2, 1, 128),
                                ap3(wv8_sb[g], 0, 512, 2, 1, 512),
                                start=(g == 0), stop=(g == 3),
                                perf_mode=DR)
                    nc.vector.tensor_copy(
                        ap3(vt8[stg >> 1], (stg & 1) * 640, 80, HPC, 1, DH),
                        ap3(ps, 0, DH, HPC, 1, DH))
                    nc.vector.memset(
                        ap3(vt8[stg >> 1], (stg & 1) * 640 + DH,
                            80, HPC, 1, 1), 1.0)

                def tail_hp(qc, hp, zt0, zt1):
                    lr = lrp.tile([1, 1024], f32, tag="lr")
                    nc.vector.tensor_copy(lr[:, 0:512], zt0[64:65, :])
                    nc.vector.tensor_copy(lr[:, 512:1024], zt1[64:65, :])
                    rr = lrp.tile([1, 1024], f32, tag="lr")
                    nc.vector.reciprocal_approx_fast(rr[:], lr[:])
                    rld = rldp.tile([1, 1024], f32, tag="rld")
                    nc.sync.dma_start(rld[:], rr[:])
                    lbs = lbsp.tile([DH, 1024], f32, tag="lbs")
                    nc.sync.dma_start(lbs[:], _bcast_ap(rld, 0, 0, DH, 1024))
                    zslice = zsum[:, qc * 512:(qc + 1) * 512]
                    for h, zt in ((0, zt0), (1, zt1)):
                        lb = lbs[:, h * 512:(h + 1) * 512]
                        if hp == 0 and h == 0:
                            nc.vector.tensor_tensor(
                                zslice, zt[0:DH, :], lb, op=ALU.mult)
                        else:
                            zn = znp.tile([DH, 512], f16, tag="zn")
                            nc.vector.tensor_tensor(
                                zn[:], zt[0:DH, :], lb, op=ALU.mult)
                            nc.vector.tensor_tensor(
                                zslice, zslice, zn[:], op=ALU.add)

                def attention0():
                    qc = 0
                    ets = {}

                    def pass1(hp):
                        lst = []
                        for kt in range(4):
                            j = kt
                            jw = j * 128
                            st2 = stq.tile([128, 1024], f32, tag="st2", name="st2")
                            nc.tensor.matmul(
                                st2[:, jw:512],
                                kt_all[0:64, hp * S + kt * 128:
                                       hp * S + (kt + 1) * 128],
                                qt_all[0:64, hp * S + jw: hp * S + 512],
                                start=True, stop=True, tile_position=(0, 0))
                            nc.tensor.matmul(
                                st2[:, 512 + jw:1024],
                                kt_all[64:128, hp * S + kt * 128:
                                       hp * S + (kt + 1) * 128],
                                qt_all[64:128, hp * S + jw: hp * S + 512],
                                start=True, stop=True, tile_position=(64, 0))
                            et = etp16.tile([128, 1024], f16, tag="et16",
                                            name="et16")
                            nc.scalar.activation(
                                ap3(et, jw, 512, 2, 1, 512 - jw),
                                ap3(st2, jw, 512, 2, 1, 512 - jw),
                                AF.Exp, scale=0.125)
                            for half in range(2):
                                blk = et[:, half * 512 + jw: half * 512 + jw + 128]
                                nc.gpsimd.affine_select(
                                    out=blk, in_=blk, compare_op=ALU.is_ge,
                                    fill=0.0, base=0, pattern=[[1, 128]],
                                    channel_multiplier=-1)
                            lst.append(et)
                        ets[hp] = lst

                    def pass2(hp):
                        zt0 = ztp.tile([65, 512], f32, tag="zt", name="zt0")
                        zt1 = ztp.tile([65, 512], f32, tag="zt", name="zt1")
                        for kt, et in enumerate(ets.pop(hp)):
                            jw = kt * 128
                            nc.tensor.matmul(
                                zt0[:, jw:512],
                                vt16[:, kt * 520 + (2 * hp) * 65:
                                     kt * 520 + (2 * hp) * 65 + 65],
                                et[:, jw:512],
                                start=(kt == 0), stop=(kt == 3))
                            nc.tensor.matmul(
                                zt1[:, jw:512],
                                vt16[:, kt * 520 + (2 * hp + 1) * 65:
                                     kt * 520 + (2 * hp + 1) * 65 + 65],
                                et[:, 512 + jw:1024],
                                start=(kt == 0), stop=(kt == 3))
                        tail_hp(qc, hp, zt0, zt1)

                    pass1(0)
                    for hp in range(4):
                        if hp < 3:
                            pass1(hp + 1)
                        pass2(hp)

                def attention(qc):
                    npairs = 2 * qc + 2
                    ets = {}

                    def pass1(hp):
                        lst = []
                        for pi in range(npairs):
                            et = etp8.tile([128, 2048], f8, tag="et8", name="et8")
                            for par in range(2):
                                kt = 2 * pi + par
                                j = kt - 4 * qc
                                jw = max(j, 0) * 128
                                st2 = stq.tile([128, 1024], f32, tag="st2",
                                               name="st2")
                                nc.tensor.matmul(
                                    st2[:, jw:512],
                                    kt_all[0:64, hp * S + kt * 128:
                                           hp * S + (kt + 1) * 128],
                                    qt_all[0:64, hp * S + qc * 512 + jw:
                                           hp * S + qc * 512 + 512],
                                    start=True, stop=True, tile_position=(0, 0))
                                nc.tensor.matmul(
                                    st2[:, 512 + jw:1024],
                                    kt_all[64:128, hp * S + kt * 128:
                                           hp * S + (kt + 1) * 128],
                                    qt_all[64:128, hp * S + qc * 512 + jw:
                                           hp * S + qc * 512 + 512],
                                    start=True, stop=True, tile_position=(64, 0))
                                nc.scalar.activation(
                                    ap3(et, par * 1024 + jw, 512, 2, 1, 512 - jw),
                                    ap3(st2, jw, 512, 2, 1, 512 - jw),
                                    AF.Exp, scale=0.125)
                                if j >= 0:
                                    for half in range(2):
                                        blk = et[:, par * 1024 + half * 512 + jw:
                                                 par * 1024 + half * 512 + jw + 128]
                                        nc.gpsimd.affine_select(
                                            out=blk, in_=blk, compare_op=ALU.is_ge,
                                            fill=0.0, base=0, pattern=[[1, 128]],
                                            channel_multiplier=-1)
                            lst.append(et)
                        ets[hp] = lst

                    def pass2(hp):
                        zt0 = ztp.tile([65, 512], f32, tag="zt", name="zt0")
                        zt1 = ztp.tile([65, 512], f32, tag="zt", name="zt1")
                        for pi, et in enumerate(ets.pop(hp)):
                            jE = 2 * pi - 4 * qc
                            if jE >= 0:
                                # diagonal pair: even-slot-only delta block as a
                                # plain fp8 matmul, DoubleRow on the shared window
                                d0, d1 = jE * 128, (jE + 1) * 128
                                for h, zt in ((0, zt0), (1, zt1)):
                                    hg = 2 * hp + h
                                    nc.tensor.matmul(
                                        zt[:, d0:d1],
                                        vt8[pi][:, hg * 80: hg * 80 + 65],
                                        et[:, h * 512 + d0: h * 512 + d1],
                                        start=False, stop=False)
                                    nc.tensor.matmul(
                                        zt[:, d1:512],
                                        ap3(vt8[pi], hg * 80, 640, 2, 1, 65),
                                        ap3(et, h * 512 + d1, 1024, 2, 1, 512 - d1),
                                        start=False, stop=(pi == npairs - 1),
                                        perf_mode=DR)
                            else:
                                for h, zt in ((0, zt0), (1, zt1)):
                                    hg = 2 * hp + h
                                    nc.tensor.matmul(
                                        zt[:, 0:512],
                                        ap3(vt8[pi], hg * 80, 640, 2, 1, 65),
                                        ap3(et, h * 512, 1024, 2, 1, 512),
                                        start=(pi == 0), stop=False,
                                        perf_mode=DR)
                        tail_hp(qc, hp, zt0, zt1)

                    pass1(0)
                    for hp in range(4):
                        if hp < 3:
                            pass1(hp + 1)
                        pass2(hp)

                def tail_proj(qc):
                    zsr = zrp.tile([128, 512], f16, tag="zsr")
                    nc.gpsimd.dma_start(zsr[DH:2 * DH, :],
                                        zsum[:, qc * 512:(qc + 1) * 512])
                    for qp in range(2):
                        for nn in range(2):
                            po0 = prp.tile([128, 512], f32, tag="pr1", name="po0")
                            po1 = prp.tile([128, 512], f32, tag="pr1", name="po1")
                            nc.tensor.matmul(
                                po0[:],
                                zsum[:, qc * 512 + (2 * qp) * 128:
                                     qc * 512 + (2 * qp + 1) * 128],
                                wo_sb[0:DH, nn * 512:(nn + 1) * 512],
                                start=True, stop=True, tile_position=(0, 0))
                            nc.tensor.matmul(
                                po1[:],
                                zsr[DH:128, (2 * qp + 1) * 128:(2 * qp + 2) * 128],
                                wo_sb[DH:128, nn * 512:(nn + 1) * 512],
                                start=True, stop=True, tile_position=(64, 0))
                            osb = osbp.tile([128, 1024], f32, tag="osb")
                            nc.vector.tensor_copy(osb[:, 0:512], po0[:])
                            nc.vector.tensor_copy(osb[:, 512:1024], po1[:])
                            r0 = qc * 512 + (2 * qp) * 128
                            nc.sync.dma_start(
                                out[r0:r0 + 128, nn * 512:(nn + 1) * 512],
                                osb[:, 0:512])
                            nc.sync.dma_start(
                                out[r0 + 128:r0 + 256, nn * 512:(nn + 1) * 512],
                                osb[:, 512:1024])

                for qc in range(NQ):
                    proj_x(qc)
                    for m in range(4):
                        proj_qk(qc, m)
                    for st in range(4):
                        proj_v(qc, st)
                    if qc == 0:
                        attention0()
                    else:
                        attention(qc)
                    if qc >= 1:
                        tail_proj(qc - 1)
                tail_proj(NQ - 1)
    nc.compile()
    return nc


def kernel(**inputs):
    import ml_dtypes
    f8 = ml_dtypes.float8_e4m3

    x = np.asarray(inputs["x"], dtype=np.float32)
    WQ = np.asarray(inputs["WQ"], dtype=np.float32)
    bQ = np.asarray(inputs["bQ"], dtype=np.float32)
    WK = np.asarray(inputs["WK"], dtype=np.float32)
    bK = np.asarray(inputs["bK"], dtype=np.float32)
    WV = np.asarray(inputs["WV"], dtype=np.float32)
    bV = np.asarray(inputs["bV"], dtype=np.float32)
    WO = np.asarray(inputs["WO"], dtype=np.float32)
    bO = np.asarray(inputs["bO"], dtype=np.float32)

    from concourse.bass_utils import run_bass_kernel_spmd

    if "nc" not in _prog:
        _prog["nc"] = _build()
    nc = _prog["nc"]

    def pair_layout(Wc):
        # [1024, 512] -> [512, 1024]: out[g*128+p, par*512+m] = Wc[g*256+par*128+p, m]
        return np.ascontiguousarray(
            Wc.reshape(4, 2, 128, GD).transpose(0, 2, 1, 3).reshape(512, 1024))

    in_maps = []
    for c in range(NCORES):
        b, g = c // 2, c % 2
        sl = slice(g * GD, (g + 1) * GD)
        xb = np.ascontiguousarray(x[b])
        in_maps.append({
            "x": xb.astype(np.float16),
            "wq8": pair_layout(WQ[:, sl]).astype(f8),
            "wk8": pair_layout(WK[:, sl]).astype(f8),
            "wv8": pair_layout(WV[:, sl]).astype(f8),
            "wv16": np.ascontiguousarray(WV[:, sl]).astype(np.float16),
            "bq": np.ascontiguousarray(bQ[sl]).reshape(1, GD).astype(np.float16),
            "bk": np.ascontiguousarray(bK[sl]).reshape(1, GD).astype(np.float16),
            "wo": WO.astype(np.float16),
        })
    _prog["in_maps"] = in_maps
    res = run_bass_kernel_spmd(nc, in_maps, core_ids=list(range(NCORES)))
    parts = [r["out"] for r in res.results]

    extra = bV.reshape(H, DH).sum(0) @ WO + np.float32(H) * bO
    out = np.empty((B, S, D), dtype=np.float32)
    for b in range(B):
        out[b] = parts[2 * b] + parts[2 * b + 1] + extra
    return out


# revision 18
# speedup vs baseline: 1.3318x; 1.0016x over previous
"""Causal self-attention kernel for 8 Trainium2 NeuronCores.

Sharding: core c -> (batch b = c//2, head-group g = c%2). Each core computes
the attention output contribution of 8 heads for one batch element:
    P_c = (sum_{h in group} softmax(Q_h K_h^T / 8 + causal) V_h) @ WO
Host epilogue: out[b] = P_{2b} + P_{2b+1} + (sum_h bV_h) @ WO + 16*bO
(the V-bias commutes through softmax normalization: softmax rows sum to 1).

Precision/speed split (PE row counts at ~1.5 G rows/s are the limiter):
  - Q/K/V projections run as fp8-E4M3 DoubleRow matmuls (K=256 per pass:
    d-tile pairs via a [128, 2, N] AP over the existing xt layout), halving
    projection PE time. Chunk 0's V is also computed in fp16 (kept in vt16)
    because output rows with tiny softmax support (q < 512) see V error
    directly; all other chunks' softmax averages 512+ values so fp8 noise
    washes out.
  - Scores stay fp16 (K=64 contraction gets no DoubleRow benefit); two heads
    per PE pass via tile_position row-tiling. Q/K fp8 projection error only
    perturbs softmax weights (benign at any support size).
  - A@V for q-chunks >= 1 runs fp8 DoubleRow over k-tile PAIRS: ET pair
    tiles [128, 2, 2head*512] written by exp directly in fp8, V pair tiles
    [128, 2, 8head*65] with a ones column accumulating the softmax
    denominator in row 64. Chunk 0 uses the fp16 per-k-tile path.
  - Diagonal k-tiles shrink the ST matmul, exp, and ZT to the unmasked
    q-window; fully masked regions are never computed (no memsets except the
    odd-slot delta of diagonal fp8 pairs). The in-block triangle is masked
    by gpsimd affine_select on ET after exp.
  - Per-hp tail: 1/l via DVE reciprocal_approx_fast (no ScalarE Ln/Exp table
    swaps), broadcast by a DRAM-bounce DMA, normalize straight out of zt
    PSUM into the zsum accumulator, so almost nothing serializes at the end.
"""
import numpy as np

B, S, D, H, DH = 4, 2048, 1024, 16, 64
HPC = 8            # heads per core
GD = HPC * DH      # 512 = group width
NCORES = 8
NQ = S // 512      # 4 q/s chunks of 512
NKT = S // 128     # 16 k-tiles
NDT = D // 128     # 8 d-tiles

_prog = {}


def ap3(tile_t, offset, d1, n1, d2, n2):
    """AP view [128p, n1, n2] over a tile's free dim: col = offset + i*d1 + j*d2."""
    import concourse.bass as bass
    ap = tile_t[:]
    return bass.AP(ap.tensor, ap.offset + offset,
                   [ap.ap[0], [d1, n1], [d2, n2]])


def bass_ap_3d(tile_t, offset, stride, n, inner):
    return ap3(tile_t, offset, stride, n, 1, inner)


def _bcast_ap(tile_t, row, col, nparts, width):
    """Partition-step-0 AP reading (row, col:col+width) replicated nparts times."""
    import concourse.bass as bass
    ap = tile_t[:]
    pstep = ap.ap[0][0]
    return bass.AP(ap.tensor, ap.offset + row * pstep + col,
                   [[0, nparts], [1, width]])


def _build():
    import concourse.bacc as bacc
    import concourse.tile as tile
    from concourse import mybir
    import concourse.bass as bass

    f32 = mybir.dt.float32
    f16 = mybir.dt.float16
    f8 = mybir.dt.float8e4
    AF = mybir.ActivationFunctionType
    ALU = mybir.AluOpType
    DR = mybir.MatmulPerfMode.DoubleRow

    nc = bacc.Bacc(None, target_bir_lowering=False, debug=False)
    x = nc.dram_tensor("x", [S, D], f16, kind="ExternalInput")
    # wq8/wk8/wv8: [4*128, 2*512]: row = g*128 + p, col = par*512 + m,
    # value = W[g*256 + par*128 + p, m]  (d-pair DoubleRow layout)
    wq8 = nc.dram_tensor("wq8", [512, 1024], f8, kind="ExternalInput")
    wk8 = nc.dram_tensor("wk8", [512, 1024], f8, kind="ExternalInput")
    wv8 = nc.dram_tensor("wv8", [512, 1024], f8, kind="ExternalInput")
    wv16 = nc.dram_tensor("wv16", [D, GD], f16, kind="ExternalInput")
    bq = nc.dram_tensor("bq", [1, GD], f16, kind="ExternalInput")
    bk = nc.dram_tensor("bk", [1, GD], f16, kind="ExternalInput")
    wo = nc.dram_tensor("wo", [DH, D], f16, kind="ExternalInput")
    out = nc.dram_tensor("out", [S, D], f32, kind="ExternalOutput")

    with tile.TileContext(nc) as tc:
        with tc.tile_pool(name="const", bufs=1) as constp, \
             tc.tile_pool(name="big", bufs=1) as bigp:
            idt16 = constp.tile([128, 128], f16, tag="idt16")
            from concourse.masks import make_identity
            make_identity(nc, idt16[:])
            bq_t = constp.tile([128, 4], f32, tag="bq_t")
            bk_t = constp.tile([128, 4], f32, tag="bk_t")
            nc.gpsimd.dma_start(bq_t[:], bass.AP(bq, 0, [[1, 128], [128, 4]]))
            nc.gpsimd.dma_start(bk_t[:], bass.AP(bk, 0, [[1, 128], [128, 4]]))
            tri2 = constp.tile([128, 256], f16, tag="tri2")
            nc.vector.memset(tri2[:], 0.0)
            for half in range(2):
                nc.gpsimd.affine_select(
                    out=tri2[:, half * 128:(half + 1) * 128],
                    in_=tri2[:, half * 128:(half + 1) * 128],
                    compare_op=ALU.is_ge, fill=-30000.0, base=0,
                    pattern=[[1, 128]], channel_multiplier=-1)
            wo_sb = constp.tile([128, D], f16, tag="wo_sb")
            nc.gpsimd.dma_start(wo_sb[0:DH, :], wo[:])
            nc.gpsimd.dma_start(wo_sb[DH:2 * DH, :], wo[:])

            # persistent per-core tensors
            xt8_all = bigp.tile([128, NDT * S], f8, tag="xt8")    # d-tile j at j*S
            xt16_c0 = bigp.tile([128, NDT * 512], f16, tag="xt16")  # d-tile j at j*512
            qt_all = bigp.tile([128, 4 * S], f16, tag="qt")       # m-tile m at m*S
            kt_all = bigp.tile([128, 4 * S], f16, tag="kt")
            # vt8: k-tile pair p: [par(2) x head(8) x 65] at col par*520+h*65+c
            vt8 = [bigp.tile([128, 1280], f8, tag=f"vt8_{p}", name=f"vt8_{p}")
                   for p in range(NKT // 2)]
            vt16 = bigp.tile([128, 4 * 520], f16, tag="vt16")     # chunk-0 k-tiles
            zsum = bigp.tile([DH, S], f16, tag="zsum")

            with tc.tile_pool(name="wts", bufs=1) as wtp, \
                 tc.tile_pool(name="xs", bufs=5) as xsp, \
                 tc.tile_pool(name="et16", bufs=6) as etp16, \
                 tc.tile_pool(name="et8", bufs=16) as etp8, \
                 tc.tile_pool(name="rld", bufs=2, space="DRAM") as rldp, \
                 tc.tile_pool(name="lr", bufs=2) as lrp, \
                 tc.tile_pool(name="lbs", bufs=2) as lbsp, \
                 tc.tile_pool(name="zn", bufs=2) as znp, \
                 tc.tile_pool(name="zr", bufs=2) as zrp, \
                 tc.tile_pool(name="osb", bufs=3) as osbp, \
                 tc.tile_pool(name="stq", bufs=2, space="PSUM") as stq, \
                 tc.tile_pool(name="prp", bufs=2, space="PSUM") as prp, \
                 tc.tile_pool(name="ztp", bufs=2, space="PSUM") as ztp:
                wq8_sb = [wtp.tile([128, 1024], f8, tag=f"wq{k}", name=f"wq{k}")
                          for k in range(4)]
                wk8_sb = [wtp.tile([128, 1024], f8, tag=f"wk{k}", name=f"wk{k}")
                          for k in range(4)]
                wv8_sb = [wtp.tile([128, 1024], f8, tag=f"wv{k}", name=f"wv{k}")
                          for k in range(4)]
                wv16_sb = [wtp.tile([128, GD], f16, tag=f"wv16_{k}",
                                    name=f"wv16_{k}") for k in range(NDT)]
                for k in range(4):
                    nc.scalar.dma_start(wq8_sb[k][:], wq8[k * 128:(k + 1) * 128, :])
                    nc.scalar.dma_start(wk8_sb[k][:], wk8[k * 128:(k + 1) * 128, :])
                    nc.scalar.dma_start(wv8_sb[k][:], wv8[k * 128:(k + 1) * 128, :])
                for k in range(NDT):
                    nc.scalar.dma_start(wv16_sb[k][:], wv16[k * 128:(k + 1) * 128, :])

                def proj_x(qc):
                    xss = []
                    for st4 in range(4):
                        srow = qc * 512 + st4 * 128
                        xs = xsp.tile([128, D], f16, tag="xs", name="xs")
                        nc.sync.dma_start(xs[:], x[srow:srow + 128, :])
                        xss.append(xs)
                    # transpose x into xT via the PE, 8 per 1-bank PSUM tile;
                    # eviction casts to fp8 (and keeps fp16 for chunk 0)
                    for jj in range(4):
                        pt = prp.tile([128, 1024], f16, tag="pr1", name="pt")
                        for j2 in range(2):
                            j = jj * 2 + j2
                            for st4 in range(4):
                                nc.tensor.transpose(
                                    pt[:, j2 * 512 + st4 * 128:
                                       j2 * 512 + (st4 + 1) * 128],
                                    xss[st4][:, j * 128:(j + 1) * 128], idt16[:])
                        dst = bass_ap_3d(xt8_all, (jj * 2) * S + qc * 512, S, 2, 512)
                        nc.vector.tensor_copy(dst, bass_ap_3d(pt, 0, 512, 2, 512))
                        if qc == 0:
                            dst16 = bass_ap_3d(xt16_c0, (jj * 2) * 512, 512, 2, 512)
                            nc.vector.tensor_copy(dst16,
                                                  bass_ap_3d(pt, 0, 512, 2, 512))
                def proj_qk(qc, m):
                    # Q/K m-tile: fp8 DoubleRow over d-pairs, 1-bank accumulators
                    for (w_sb, b_t, dest) in ((wq8_sb, bq_t, qt_all),
                                              (wk8_sb, bk_t, kt_all)):
                        ps = prp.tile([128, 512], f32, tag="pr1", name="ps")
                        for g in range(4):
                            nc.tensor.matmul(
                                ps[:],
                                ap3(w_sb[g], m * 128, 512, 2, 1, 128),
                                ap3(xt8_all, (2 * g) * S + qc * 512,
                                    S, 2, 1, 512),
                                start=(g == 0), stop=(g == 3),
                                perf_mode=DR)
                        nc.vector.tensor_scalar_add(
                            dest[:, m * S + qc * 512: m * S + (qc + 1) * 512],
                            ps[:], b_t[:, m:m + 1])

                def proj_v(qc, st):
                    stg = qc * 4 + st
                    ps = prp.tile([128, 512], f32, tag="pr1", name="ps")
                    if qc == 0:
                        for k in range(NDT):
                            nc.tensor.matmul(
                                ps[:],
                                xt16_c0[:, k * 512 + st * 128:
                                        k * 512 + (st + 1) * 128],
                                wv16_sb[k][:],
                                start=(k == 0), stop=(k == NDT - 1))
                        src = ap3(ps, 0, DH, HPC, 1, DH)
                        nc.vector.tensor_copy(
                            ap3(vt16, st * 520, 65, HPC, 1, DH), src)
                        nc.vector.memset(
                            ap3(vt16, st * 520 + DH, 65, HPC, 1, 1), 1.0)
                    else:
                        for g in range(4):
                            nc.tensor.matmul(
                                ps[:],
                                ap3(xt8_all, (2 * g) * S + qc * 512 + st * 128,
                                    S, 2, 1, 128),
                                ap3(wv8_sb[g], 0, 512, 2, 1, 512),
                                start=(g == 0), stop=(g == 3),
                                perf_mode=DR)
                    nc.vector.tensor_copy(
                        ap3(vt8[stg >> 1], (stg & 1) * 640, 80, HPC, 1, DH),
                        ap3(ps, 0, DH, HPC, 1, DH))
                    nc.vector.memset(
                        ap3(vt8[stg >> 1], (stg & 1) * 640 + DH,
                            80, HPC, 1, 1), 1.0)

                def tail_hp(qc, hp, zt0, zt1):
                    lr = lrp.tile([1, 1024], f32, tag="lr")
                    nc.vector.tensor_copy(lr[:, 0:512], zt0[64:65, :])
                    nc.vector.tensor_copy(lr[:, 512:1024], zt1[64:65, :])
                    rr = lrp.tile([1, 1024], f32, tag="lr")
                    nc.vector.reciprocal_approx_fast(rr[:], lr[:])
                    rld = rldp.tile([1, 1024], f32, tag="rld")
                    nc.sync.dma_start(rld[:], rr[:])
                    lbs = lbsp.tile([DH, 1024], f32, tag="lbs")
                    nc.sync.dma_start(lbs[:], _bcast_ap(rld, 0, 0, DH, 1024))
                    zslice = zsum[:, qc * 512:(qc + 1) * 512]
                    for h, zt in ((0, zt0), (1, zt1)):
                        lb = lbs[:, h * 512:(h + 1) * 512]
                        if hp == 0 and h == 0:
                            nc.vector.tensor_tensor(
                                zslice, zt[0:DH, :], lb, op=ALU.mult)
                        else:
                            zn = znp.tile([DH, 512], f16, tag="zn")
                            nc.vector.tensor_tensor(
                                zn[:], zt[0:DH, :], lb, op=ALU.mult)
                            nc.gpsimd.tensor_tensor(
                                zslice, zslice, zn[:], op=ALU.add)

                def attention0():
                    qc = 0
                    ets = {}

                    def pass1(hp):
                        lst = []
                        for kt in range(4):
                            j = kt
                            jw = j * 128
                            st2 = stq.tile([128, 1024], f32, tag="st2", name="st2")
                            nc.tensor.matmul(
                                st2[:, jw:512],
                                kt_all[0:64, hp * S + kt * 128:
                                       hp * S + (kt + 1) * 128],
                                qt_all[0:64, hp * S + jw: hp * S + 512],
                                start=True, stop=False, tile_position=(0, 0))
                            nc.tensor.matmul(
                                st2[:, 512 + jw:1024],
                                kt_all[64:128, hp * S + kt * 128:
                                       hp * S + (kt + 1) * 128],
                                qt_all[64:128, hp * S + jw: hp * S + 512],
                                start=True, stop=False, tile_position=(64, 0))
                            nc.tensor.matmul(
                                ap3(st2, jw, 512, 2, 1, 128),
                                idt16[:], tri2[:],
                                start=False, stop=True)
                            et = etp16.tile([128, 1024], f16, tag="et16",
                                            name="et16")
                            nc.scalar.activation(
                                ap3(et, jw, 512, 2, 1, 512 - jw),
                                ap3(st2, jw, 512, 2, 1, 512 - jw),
                                AF.Exp, scale=0.125)
                            lst.append(et)
                        ets[hp] = lst

                    def pass2(hp):
                        zt0 = ztp.tile([65, 512], f32, tag="zt", name="zt0")
                        zt1 = ztp.tile([65, 512], f32, tag="zt", name="zt1")
                        for kt, et in enumerate(ets.pop(hp)):
                            jw = kt * 128
                            nc.tensor.matmul(
                                zt0[:, jw:512],
                                vt16[:, kt * 520 + (2 * hp) * 65:
                                     kt * 520 + (2 * hp) * 65 + 65],
                                et[:, jw:512],
                                start=(kt == 0), stop=(kt == 3))
                            nc.tensor.matmul(
                                zt1[:, jw:512],
                                vt16[:, kt * 520 + (2 * hp + 1) * 65:
                                     kt * 520 + (2 * hp + 1) * 65 + 65],
                                et[:, 512 + jw:1024],
                                start=(kt == 0), stop=(kt == 3))
                        tail_hp(qc, hp, zt0, zt1)

                    pass1(0)
                    for hp in range(4):
                        if hp < 3:
                            pass1(hp + 1)
                        pass2(hp)

                def attention(qc):
                    npairs = 2 * qc + 2
                    ets = {}

                    def pass1(hp):
                        lst = []
                        for pi in range(npairs):
                            et = etp8.tile([128, 2048], f8, tag="et8", name="et8")
                            for par in range(2):
                                kt = 2 * pi + par
                                j = kt - 4 * qc
                                jw = max(j, 0) * 128
                                st2 = stq.tile([128, 1024], f32, tag="st2",
                                               name="st2")
                                diag = j >= 0
                                nc.tensor.matmul(
                                    st2[:, jw:512],
                                    kt_all[0:64, hp * S + kt * 128:
                                           hp * S + (kt + 1) * 128],
                                    qt_all[0:64, hp * S + qc * 512 + jw:
                                           hp * S + qc * 512 + 512],
                                    start=True, stop=not diag,
                                    tile_position=(0, 0))
                                nc.tensor.matmul(
                                    st2[:, 512 + jw:1024],
                                    kt_all[64:128, hp * S + kt * 128:
                                           hp * S + (kt + 1) * 128],
                                    qt_all[64:128, hp * S + qc * 512 + jw:
                                           hp * S + qc * 512 + 512],
                                    start=True, stop=not diag,
                                    tile_position=(64, 0))
                                if diag:
                                    nc.tensor.matmul(
                                        ap3(st2, jw, 512, 2, 1, 128),
                                        idt16[:], tri2[:],
                                        start=False, stop=True)
                                nc.scalar.activation(
                                    ap3(et, par * 1024 + jw, 512, 2, 1, 512 - jw),
                                    ap3(st2, jw, 512, 2, 1, 512 - jw),
                                    AF.Exp, scale=0.125)
                            lst.append(et)
                        ets[hp] = lst

                    def pass2(hp):
                        zt0 = ztp.tile([65, 512], f32, tag="zt", name="zt0")
                        zt1 = ztp.tile([65, 512], f32, tag="zt", name="zt1")
                        for pi, et in enumerate(ets.pop(hp)):
                            jE = 2 * pi - 4 * qc
                            if jE >= 0:
                                # diagonal pair: even-slot-only delta block as a
                                # plain fp8 matmul, DoubleRow on the shared window
                                d0, d1 = jE * 128, (jE + 1) * 128
                                for h, zt in ((0, zt0), (1, zt1)):
                                    hg = 2 * hp + h
                                    nc.tensor.matmul(
                                        zt[:, d0:d1],
                                        vt8[pi][:, hg * 80: hg * 80 + 65],
                                        et[:, h * 512 + d0: h * 512 + d1],
                                        start=False, stop=False)
                                    nc.tensor.matmul(
                                        zt[:, d1:512],
                                        ap3(vt8[pi], hg * 80, 640, 2, 1, 65),
                                        ap3(et, h * 512 + d1, 1024, 2, 1, 512 - d1),
                                        start=False, stop=(pi == npairs - 1),
                                        perf_mode=DR)
                            else:
                                for h, zt in ((0, zt0), (1, zt1)):
                                    hg = 2 * hp + h
                                    nc.tensor.matmul(
                                        zt[:, 0:512],
                                        ap3(vt8[pi], hg * 80, 640, 2, 1, 65),
                                        ap3(et, h * 512, 1024, 2, 1, 512),
                                        start=(pi == 0), stop=False,
                                        perf_mode=DR)
                        tail_hp(qc, hp, zt0, zt1)

                    pass1(0)
                    for hp in range(4):
                        if hp < 3:
                            pass1(hp + 1)
                        pass2(hp)

                def tail_proj(qc):
                    zsr = zrp.tile([128, 512], f16, tag="zsr")
                    nc.gpsimd.dma_start(zsr[DH:2 * DH, :],
                                        zsum[:, qc * 512:(qc + 1) * 512])
                    for qp in range(2):
                        for nn in range(2):
                            po0 = prp.tile([128, 512], f32, tag="pr1", name="po0")
                            po1 = prp.tile([128, 512], f32, tag="pr1", name="po1")
                            nc.tensor.matmul(
                                po0[:],
                                zsum[:, qc * 512 + (2 * qp) * 128:
                                     qc * 512 + (2 * qp + 1) * 128],
                                wo_sb[0:DH, nn * 512:(nn + 1) * 512],
                                start=True, stop=True, tile_position=(0, 0))
                            nc.tensor.matmul(
                                po1[:],
                                zsr[DH:128, (2 * qp + 1) * 128:(2 * qp + 2) * 128],
                                wo_sb[DH:128, nn * 512:(nn + 1) * 512],
                                start=True, stop=True, tile_position=(64, 0))
                            osb = osbp.tile([128, 1024], f32, tag="osb")
                            nc.vector.tensor_copy(osb[:, 0:512], po0[:])
                            nc.vector.tensor_copy(osb[:, 512:1024], po1[:])
                            r0 = qc * 512 + (2 * qp) * 128
                            nc.sync.dma_start(
                                out[r0:r0 + 128, nn * 512:(nn + 1) * 512],
                                osb[:, 0:512])
                            nc.sync.dma_start(
                                out[r0 + 128:r0 + 256, nn * 512:(nn + 1) * 512],
                                osb[:, 512:1024])

                for qc in range(NQ):
                    proj_x(qc)
                    for m in range(4):
                        proj_qk(qc, m)
                    for st in range(4):
                        proj_v(qc, st)
                    if qc == 0:
                        attention0()
                    else:
                        attention(qc)
                    if qc >= 1:
                        tail_proj(qc - 1)
                tail_proj(NQ - 1)
    nc.compile()
    return nc


def kernel(**inputs):
    import ml_dtypes
    f8 = ml_dtypes.float8_e4m3

    x = np.asarray(inputs["x"], dtype=np.float32)
    WQ = np.asarray(inputs["WQ"], dtype=np.float32)
    bQ = np.asarray(inputs["bQ"], dtype=np.float32)
    WK = np.asarray(inputs["WK"], dtype=np.float32)
    bK = np.asarray(inputs["bK"], dtype=np.float32)
    WV = np.asarray(inputs["WV"], dtype=np.float32)
    bV = np.asarray(inputs["bV"], dtype=np.float32)
    WO = np.asarray(inputs["WO"], dtype=np.float32)
    bO = np.asarray(inputs["bO"], dtype=np.float32)

    from concourse.bass_utils import run_bass_kernel_spmd

    if "nc" not in _prog:
        _prog["nc"] = _build()
    nc = _prog["nc"]

    def pair_layout(Wc):
        # [1024, 512] -> [512, 1024]: out[g*128+p, par*512+m] = Wc[g*256+par*128+p, m]
        return np.ascontiguousarray(
            Wc.reshape(4, 2, 128, GD).transpose(0, 2, 1, 3).reshape(512, 1024))

    in_maps = []
    for c in range(NCORES):
        b, g = c // 2, c % 2
        sl = slice(g * GD, (g + 1) * GD)
        xb = np.ascontiguousarray(x[b])
        in_maps.append({
            "x": xb.astype(np.float16),
            "wq8": pair_layout(WQ[:, sl]).astype(f8),
            "wk8": pair_layout(WK[:, sl]).astype(f8),
            "wv8": pair_layout(WV[:, sl]).astype(f8),
            "wv16": np.ascontiguousarray(WV[:, sl]).astype(np.float16),
            "bq": np.ascontiguousarray(bQ[sl]).reshape(1, GD).astype(np.float16),
            "bk": np.ascontiguousarray(bK[sl]).reshape(1, GD).astype(np.float16),
            "wo": WO.astype(np.float16),
        })
    _prog["in_maps"] = in_maps
    res = run_bass_kernel_spmd(nc, in_maps, core_ids=list(range(NCORES)))
    parts = [r["out"] for r in res.results]

    extra = bV.reshape(H, DH).sum(0) @ WO + np.float32(H) * bO
    out = np.empty((B, S, D), dtype=np.float32)
    for b in range(B):
        out[b] = parts[2 * b] + parts[2 * b + 1] + extra
    return out


# revision 19
# speedup vs baseline: 1.3363x; 1.0034x over previous
"""Causal self-attention kernel for 8 Trainium2 NeuronCores.

Sharding: core c -> (batch b = c//2, head-group g = c%2). Each core computes
the attention output contribution of 8 heads for one batch element:
    P_c = (sum_{h in group} softmax(Q_h K_h^T / 8 + causal) V_h) @ WO
Host epilogue: out[b] = P_{2b} + P_{2b+1} + (sum_h bV_h) @ WO + 16*bO
(the V-bias commutes through softmax normalization: softmax rows sum to 1).

Precision/speed split (PE row counts at ~1.5 G rows/s are the limiter):
  - Q/K/V projections run as fp8-E4M3 DoubleRow matmuls (K=256 per pass:
    d-tile pairs via a [128, 2, N] AP over the existing xt layout), halving
    projection PE time. Chunk 0's V is also computed in fp16 (kept in vt16)
    because output rows with tiny softmax support (q < 512) see V error
    directly; all other chunks' softmax averages 512+ values so fp8 noise
    washes out.
  - Scores stay fp16 (K=64 contraction gets no DoubleRow benefit); two heads
    per PE pass via tile_position row-tiling. Q/K fp8 projection error only
    perturbs softmax weights (benign at any support size).
  - A@V for q-chunks >= 1 runs fp8 DoubleRow over k-tile PAIRS: ET pair
    tiles [128, 2, 2head*512] written by exp directly in fp8, V pair tiles
    [128, 2, 8head*65] with a ones column accumulating the softmax
    denominator in row 64. Chunk 0 uses the fp16 per-k-tile path.
  - Diagonal k-tiles shrink the ST matmul, exp, and ZT to the unmasked
    q-window; fully masked regions are never computed. The in-block causal
    triangle is applied by accumulating a constant -30000 upper-triangle
    into the ST PSUM via one extra PE matmul (identity @ tri), so exp yields
    exact zeros and no gpsimd affine_select sits on the exp->ZT chain. The
    odd k-tile of a diagonal fp8 pair is handled by splitting A@V into a
    plain fp8 matmul on the even-only 128-q block plus DoubleRow on the
    shared window.
  - Scheduling: attention is emitted per head-pair as pass1 (all ST+exp,
    buffered in a 16-deep fp8 ET pool) then pass2 (all A@V + tail), with
    pass1(hp+1) emitted before pass2(hp) so ScalarE sees a continuous exp
    stream and A@V/projection matmuls act as PE filler. PSUM is split into
    dedicated pools (ST 2x2 banks, proj/out 2x1 bank, Z-accum 2x1 bank) so
    next-chunk projections never contend with score tiles for banks.
  - Per-hp tail: 1/l via DVE reciprocal_approx_fast (no ScalarE Ln/Exp table
    swaps), broadcast by a DRAM-bounce DMA, normalize straight out of zt
    PSUM into a f16 zsum accumulator (mult on DVE, add on gpsimd) that the
    out-projection uses directly as its stationary operand.
  - DMA queues: x tiles on sync, all weights on the scalar queue, so neither
    the gpsimd queue nor the x loads are blocked at startup.
"""
import numpy as np

B, S, D, H, DH = 4, 2048, 1024, 16, 64
HPC = 8            # heads per core
GD = HPC * DH      # 512 = group width
NCORES = 8
NQ = S // 512      # 4 q/s chunks of 512
NKT = S // 128     # 16 k-tiles
NDT = D // 128     # 8 d-tiles

_prog = {}


def ap3(tile_t, offset, d1, n1, d2, n2):
    """AP view [128p, n1, n2] over a tile's free dim: col = offset + i*d1 + j*d2."""
    import concourse.bass as bass
    ap = tile_t[:]
    return bass.AP(ap.tensor, ap.offset + offset,
                   [ap.ap[0], [d1, n1], [d2, n2]])


def bass_ap_3d(tile_t, offset, stride, n, inner):
    return ap3(tile_t, offset, stride, n, 1, inner)


def _bcast_ap(tile_t, row, col, nparts, width):
    """Partition-step-0 AP reading (row, col:col+width) replicated nparts times."""
    import concourse.bass as bass
    ap = tile_t[:]
    pstep = ap.ap[0][0]
    return bass.AP(ap.tensor, ap.offset + row * pstep + col,
                   [[0, nparts], [1, width]])


def _build():
    import concourse.bacc as bacc
    import concourse.tile as tile
    from concourse import mybir
    import concourse.bass as bass

    f32 = mybir.dt.float32
    f16 = mybir.dt.float16
    f8 = mybir.dt.float8e4
    AF = mybir.ActivationFunctionType
    ALU = mybir.AluOpType
    DR = mybir.MatmulPerfMode.DoubleRow

    nc = bacc.Bacc(None, target_bir_lowering=False, debug=False)
    x = nc.dram_tensor("x", [S, D], f16, kind="ExternalInput")
    # wq8/wk8/wv8: [4*128, 2*512]: row = g*128 + p, col = par*512 + m,
    # value = W[g*256 + par*128 + p, m]  (d-pair DoubleRow layout)
    wq8 = nc.dram_tensor("wq8", [512, 1024], f8, kind="ExternalInput")
    wk8 = nc.dram_tensor("wk8", [512, 1024], f8, kind="ExternalInput")
    wv8 = nc.dram_tensor("wv8", [512, 1024], f8, kind="ExternalInput")
    wv16 = nc.dram_tensor("wv16", [D, GD], f16, kind="ExternalInput")
    bq = nc.dram_tensor("bq", [1, GD], f16, kind="ExternalInput")
    bk = nc.dram_tensor("bk", [1, GD], f16, kind="ExternalInput")
    wo = nc.dram_tensor("wo", [DH, D], f16, kind="ExternalInput")
    out = nc.dram_tensor("out", [S, D], f32, kind="ExternalOutput")

    with tile.TileContext(nc) as tc:
        with tc.tile_pool(name="const", bufs=1) as constp, \
             tc.tile_pool(name="big", bufs=1) as bigp:
            idt16 = constp.tile([128, 128], f16, tag="idt16")
            from concourse.masks import make_identity
            make_identity(nc, idt16[:])
            bq_t = constp.tile([128, 4], f32, tag="bq_t")
            bk_t = constp.tile([128, 4], f32, tag="bk_t")
            nc.gpsimd.dma_start(bq_t[:], bass.AP(bq, 0, [[1, 128], [128, 4]]))
            nc.gpsimd.dma_start(bk_t[:], bass.AP(bk, 0, [[1, 128], [128, 4]]))
            tri2 = constp.tile([128, 256], f16, tag="tri2")
            nc.vector.memset(tri2[:], 0.0)
            for half in range(2):
                nc.gpsimd.affine_select(
                    out=tri2[:, half * 128:(half + 1) * 128],
                    in_=tri2[:, half * 128:(half + 1) * 128],
                    compare_op=ALU.is_ge, fill=-30000.0, base=0,
                    pattern=[[1, 128]], channel_multiplier=-1)
            wo_sb = constp.tile([128, D], f16, tag="wo_sb")
            nc.gpsimd.dma_start(wo_sb[0:DH, :], wo[:])
            nc.gpsimd.dma_start(wo_sb[DH:2 * DH, :], wo[:])

            # persistent per-core tensors
            xt8_all = bigp.tile([128, NDT * S], f8, tag="xt8")    # d-tile j at j*S
            xt16_c0 = bigp.tile([128, NDT * 512], f16, tag="xt16")  # d-tile j at j*512
            qt_all = bigp.tile([128, 4 * S], f16, tag="qt")       # m-tile m at m*S
            kt_all = bigp.tile([128, 4 * S], f16, tag="kt")
            # vt8: k-tile pair p: [par(2) x head(8) x 65] at col par*520+h*65+c
            vt8 = [bigp.tile([128, 1280], f8, tag=f"vt8_{p}", name=f"vt8_{p}")
                   for p in range(NKT // 2)]
            vt16 = bigp.tile([128, 4 * 520], f16, tag="vt16")     # chunk-0 k-tiles
            zsum = bigp.tile([DH, S], f16, tag="zsum")

            with tc.tile_pool(name="wts", bufs=1) as wtp, \
                 tc.tile_pool(name="xs", bufs=5) as xsp, \
                 tc.tile_pool(name="et16", bufs=6) as etp16, \
                 tc.tile_pool(name="et8", bufs=16) as etp8, \
                 tc.tile_pool(name="rld", bufs=2, space="DRAM") as rldp, \
                 tc.tile_pool(name="lr", bufs=2) as lrp, \
                 tc.tile_pool(name="lbs", bufs=2) as lbsp, \
                 tc.tile_pool(name="zn", bufs=2) as znp, \
                 tc.tile_pool(name="zr", bufs=2) as zrp, \
                 tc.tile_pool(name="osb", bufs=3) as osbp, \
                 tc.tile_pool(name="stq", bufs=2, space="PSUM") as stq, \
                 tc.tile_pool(name="prp", bufs=2, space="PSUM") as prp, \
                 tc.tile_pool(name="ztp", bufs=2, space="PSUM") as ztp:
                wq8_sb = [wtp.tile([128, 1024], f8, tag=f"wq{k}", name=f"wq{k}")
                          for k in range(4)]
                wk8_sb = [wtp.tile([128, 1024], f8, tag=f"wk{k}", name=f"wk{k}")
                          for k in range(4)]
                wv8_sb = [wtp.tile([128, 1024], f8, tag=f"wv{k}", name=f"wv{k}")
                          for k in range(4)]
                wv16_sb = [wtp.tile([128, GD], f16, tag=f"wv16_{k}",
                                    name=f"wv16_{k}") for k in range(NDT)]
                for k in range(4):
                    nc.scalar.dma_start(wq8_sb[k][:], wq8[k * 128:(k + 1) * 128, :])
                    nc.scalar.dma_start(wk8_sb[k][:], wk8[k * 128:(k + 1) * 128, :])
                    nc.scalar.dma_start(wv8_sb[k][:], wv8[k * 128:(k + 1) * 128, :])
                for k in range(NDT):
                    nc.scalar.dma_start(wv16_sb[k][:], wv16[k * 128:(k + 1) * 128, :])

                def proj_x(qc):
                    xss = []
                    for st4 in range(4):
                        srow = qc * 512 + st4 * 128
                        xs = xsp.tile([128, D], f16, tag="xs", name="xs")
                        nc.sync.dma_start(xs[:], x[srow:srow + 128, :])
                        xss.append(xs)
                    # transpose x into xT via the PE, 8 per 1-bank PSUM tile;
                    # eviction casts to fp8 (and keeps fp16 for chunk 0)
                    for jj in range(4):
                        pt = prp.tile([128, 1024], f16, tag="pr1", name="pt")
                        for j2 in range(2):
                            j = jj * 2 + j2
                            for st4 in range(4):
                                nc.tensor.transpose(
                                    pt[:, j2 * 512 + st4 * 128:
                                       j2 * 512 + (st4 + 1) * 128],
                                    xss[st4][:, j * 128:(j + 1) * 128], idt16[:])
                        dst = bass_ap_3d(xt8_all, (jj * 2) * S + qc * 512, S, 2, 512)
                        nc.vector.tensor_copy(dst, bass_ap_3d(pt, 0, 512, 2, 512))
                        if qc == 0:
                            dst16 = bass_ap_3d(xt16_c0, (jj * 2) * 512, 512, 2, 512)
                            nc.vector.tensor_copy(dst16,
                                                  bass_ap_3d(pt, 0, 512, 2, 512))
                def proj_qk(qc, m):
                    # Q/K m-tile: fp8 DoubleRow over d-pairs, 1-bank accumulators
                    for (w_sb, b_t, dest) in ((wq8_sb, bq_t, qt_all),
                                              (wk8_sb, bk_t, kt_all)):
                        ps = prp.tile([128, 512], f32, tag="pr1", name="ps")
                        for g in range(4):
                            nc.tensor.matmul(
                                ps[:],
                                ap3(w_sb[g], m * 128, 512, 2, 1, 128),
                                ap3(xt8_all, (2 * g) * S + qc * 512,
                                    S, 2, 1, 512),
                                start=(g == 0), stop=(g == 3),
                                perf_mode=DR)
                        nc.vector.tensor_scalar_add(
                            dest[:, m * S + qc * 512: m * S + (qc + 1) * 512],
                            ps[:], b_t[:, m:m + 1])

                def proj_v(qc, st):
                    stg = qc * 4 + st
                    ps = prp.tile([128, 512], f32, tag="pr1", name="ps")
                    if qc == 0:
                        for k in range(NDT):
                            nc.tensor.matmul(
                                ps[:],
                                xt16_c0[:, k * 512 + st * 128:
                                        k * 512 + (st + 1) * 128],
                                wv16_sb[k][:],
                                start=(k == 0), stop=(k == NDT - 1))
                        src = ap3(ps, 0, DH, HPC, 1, DH)
                        nc.vector.tensor_copy(
                            ap3(vt16, st * 520, 65, HPC, 1, DH), src)
                        nc.vector.memset(
                            ap3(vt16, st * 520 + DH, 65, HPC, 1, 1), 1.0)
                    else:
                        for g in range(4):
                            nc.tensor.matmul(
                                ps[:],
                                ap3(xt8_all, (2 * g) * S + qc * 512 + st * 128,
                                    S, 2, 1, 128),
                                ap3(wv8_sb[g], 0, 512, 2, 1, 512),
                                start=(g == 0), stop=(g == 3),
                                perf_mode=DR)
                    nc.vector.tensor_copy(
                        ap3(vt8[stg >> 1], (stg & 1) * 640, 80, HPC, 1, DH),
                        ap3(ps, 0, DH, HPC, 1, DH))
                    nc.vector.memset(
                        ap3(vt8[stg >> 1], (stg & 1) * 640 + DH,
                            80, HPC, 1, 1), 1.0)

                def tail_hp(qc, hp, zt0, zt1):
                    lr = lrp.tile([1, 1024], f32, tag="lr")
                    nc.vector.tensor_copy(lr[:, 0:512], zt0[64:65, :])
                    nc.vector.tensor_copy(lr[:, 512:1024], zt1[64:65, :])
                    rr = lrp.tile([1, 1024], f32, tag="lr")
                    nc.vector.reciprocal_approx_fast(rr[:], lr[:])
                    rld = rldp.tile([1, 1024], f32, tag="rld")
                    nc.sync.dma_start(rld[:], rr[:])
                    lbs = lbsp.tile([DH, 1024], f32, tag="lbs")
                    nc.sync.dma_start(lbs[:], _bcast_ap(rld, 0, 0, DH, 1024))
                    zslice = zsum[:, qc * 512:(qc + 1) * 512]
                    for h, zt in ((0, zt0), (1, zt1)):
                        lb = lbs[:, h * 512:(h + 1) * 512]
                        if hp == 0 and h == 0:
                            nc.vector.tensor_tensor(
                                zslice, zt[0:DH, :], lb, op=ALU.mult)
                        else:
                            zn = znp.tile([DH, 512], f16, tag="zn")
                            nc.vector.tensor_tensor(
                                zn[:], zt[0:DH, :], lb, op=ALU.mult)
                            nc.gpsimd.tensor_tensor(
                                zslice, zslice, zn[:], op=ALU.add)

                def attention0():
                    qc = 0
                    ets = {}

                    def pass1(hp):
                        lst = []
                        for kt in range(4):
                            j = kt
                            jw = j * 128
                            st2 = stq.tile([128, 1024], f32, tag="st2", name="st2")
                            nc.tensor.matmul(
                                st2[:, jw:512],
                                kt_all[0:64, hp * S + kt * 128:
                                       hp * S + (kt + 1) * 128],
                                qt_all[0:64, hp * S + jw: hp * S + 512],
                                start=True, stop=False, tile_position=(0, 0))
                            nc.tensor.matmul(
                                st2[:, 512 + jw:1024],
                                kt_all[64:128, hp * S + kt * 128:
                                       hp * S + (kt + 1) * 128],
                                qt_all[64:128, hp * S + jw: hp * S + 512],
                                start=True, stop=False, tile_position=(64, 0))
                            nc.tensor.matmul(
                                ap3(st2, jw, 512, 2, 1, 128),
                                idt16[:], tri2[:],
                                start=False, stop=True)
                            et = etp16.tile([128, 1024], f16, tag="et16",
                                            name="et16")
                            nc.scalar.activation(
                                ap3(et, jw, 512, 2, 1, 512 - jw),
                                ap3(st2, jw, 512, 2, 1, 512 - jw),
                                AF.Exp, scale=0.125)
                            lst.append(et)
                        ets[hp] = lst

                    def pass2(hp):
                        zt0 = ztp.tile([65, 512], f32, tag="zt", name="zt0")
                        zt1 = ztp.tile([65, 512], f32, tag="zt", name="zt1")
                        for kt, et in enumerate(ets.pop(hp)):
                            jw = kt * 128
                            nc.tensor.matmul(
                                zt0[:, jw:512],
                                vt16[:, kt * 520 + (2 * hp) * 65:
                                     kt * 520 + (2 * hp) * 65 + 65],
                                et[:, jw:512],
                                start=(kt == 0), stop=(kt == 3))
                            nc.tensor.matmul(
                                zt1[:, jw:512],
                                vt16[:, kt * 520 + (2 * hp + 1) * 65:
                                     kt * 520 + (2 * hp + 1) * 65 + 65],
                                et[:, 512 + jw:1024],
                                start=(kt == 0), stop=(kt == 3))
                        tail_hp(qc, hp, zt0, zt1)

                    pass1(0)
                    for hp in range(4):
                        if hp < 3:
                            pass1(hp + 1)
                        pass2(hp)

                def attention(qc):
                    npairs = 2 * qc + 2
                    ets = {}

                    def pass1(hp):
                        lst = []
                        for pi in range(npairs):
                            et = etp8.tile([128, 2048], f8, tag="et8", name="et8")
                            for par in range(2):
                                kt = 2 * pi + par
                                j = kt - 4 * qc
                                jw = max(j, 0) * 128
                                st2 = stq.tile([128, 1024], f32, tag="st2",
                                               name="st2")
                                diag = j >= 0
                                nc.tensor.matmul(
                                    st2[:, jw:512],
                                    kt_all[0:64, hp * S + kt * 128:
                                           hp * S + (kt + 1) * 128],
                                    qt_all[0:64, hp * S + qc * 512 + jw:
                                           hp * S + qc * 512 + 512],
                                    start=True, stop=not diag,
                                    tile_position=(0, 0))
                                nc.tensor.matmul(
                                    st2[:, 512 + jw:1024],
                                    kt_all[64:128, hp * S + kt * 128:
                                           hp * S + (kt + 1) * 128],
                                    qt_all[64:128, hp * S + qc * 512 + jw:
                                           hp * S + qc * 512 + 512],
                                    start=True, stop=not diag,
                                    tile_position=(64, 0))
                                if diag:
                                    nc.tensor.matmul(
                                        ap3(st2, jw, 512, 2, 1, 128),
                                        idt16[:], tri2[:],
                                        start=False, stop=True)
                                nc.scalar.activation(
                                    ap3(et, par * 1024 + jw, 512, 2, 1, 512 - jw),
                                    ap3(st2, jw, 512, 2, 1, 512 - jw),
                                    AF.Exp, scale=0.125)
                            lst.append(et)
                        ets[hp] = lst

                    def pass2(hp):
                        zt0 = ztp.tile([65, 512], f32, tag="zt", name="zt0")
                        zt1 = ztp.tile([65, 512], f32, tag="zt", name="zt1")
                        for pi, et in enumerate(ets.pop(hp)):
                            jE = 2 * pi - 4 * qc
                            if jE >= 0:
                                # diagonal pair: even-slot-only delta block as a
                                # plain fp8 matmul, DoubleRow on the shared window
                                d0, d1 = jE * 128, (jE + 1) * 128
                                for h, zt in ((0, zt0), (1, zt1)):
                                    hg = 2 * hp + h
                                    nc.tensor.matmul(
                                        zt[:, d0:d1],
                                        vt8[pi][:, hg * 80: hg * 80 + 65],
                                        et[:, h * 512 + d0: h * 512 + d1],
                                        start=False, stop=False)
                                    nc.tensor.matmul(
                                        zt[:, d1:512],
                                        ap3(vt8[pi], hg * 80, 640, 2, 1, 65),
                                        ap3(et, h * 512 + d1, 1024, 2, 1, 512 - d1),
                                        start=False, stop=(pi == npairs - 1),
                                        perf_mode=DR)
                            else:
                                for h, zt in ((0, zt0), (1, zt1)):
                                    hg = 2 * hp + h
                                    nc.tensor.matmul(
                                        zt[:, 0:512],
                                        ap3(vt8[pi], hg * 80, 640, 2, 1, 65),
                                        ap3(et, h * 512, 1024, 2, 1, 512),
                                        start=(pi == 0), stop=False,
                                        perf_mode=DR)
                        tail_hp(qc, hp, zt0, zt1)

                    pass1(0)
                    for hp in range(4):
                        if hp < 3:
                            pass1(hp + 1)
                        pass2(hp)

                def tail_proj(qc):
                    zsr = zrp.tile([128, 512], f16, tag="zsr")
                    nc.gpsimd.dma_start(zsr[DH:2 * DH, :],
                                        zsum[:, qc * 512:(qc + 1) * 512])
                    for qp in range(2):
                        for nn in range(2):
                            po0 = prp.tile([128, 512], f32, tag="pr1", name="po0")
                            po1 = prp.tile([128, 512], f32, tag="pr1", name="po1")
                            nc.tensor.matmul(
                                po0[:],
                                zsum[:, qc * 512 + (2 * qp) * 128:
                                     qc * 512 + (2 * qp + 1) * 128],
                                wo_sb[0:DH, nn * 512:(nn + 1) * 512],
                                start=True, stop=True, tile_position=(0, 0))
                            nc.tensor.matmul(
                                po1[:],
                                zsr[DH:128, (2 * qp + 1) * 128:(2 * qp + 2) * 128],
                                wo_sb[DH:128, nn * 512:(nn + 1) * 512],
                                start=True, stop=True, tile_position=(64, 0))
                            osb = osbp.tile([128, 1024], f32, tag="osb")
                            nc.vector.tensor_copy(osb[:, 0:512], po0[:])
                            nc.vector.tensor_copy(osb[:, 512:1024], po1[:])
                            r0 = qc * 512 + (2 * qp) * 128
                            nc.sync.dma_start(
                                out[r0:r0 + 128, nn * 512:(nn + 1) * 512],
                                osb[:, 0:512])
                            nc.sync.dma_start(
                                out[r0 + 128:r0 + 256, nn * 512:(nn + 1) * 512],
                                osb[:, 512:1024])

                for qc in range(NQ):
                    proj_x(qc)
                    for m in range(4):
                        proj_qk(qc, m)
                    for st in range(4):
                        proj_v(qc, st)
                    if qc == 0:
                        attention0()
                    else:
                        attention(qc)
                    if qc >= 1:
                        tail_proj(qc - 1)
                tail_proj(NQ - 1)
    nc.compile()
    return nc


def kernel(**inputs):
    import ml_dtypes
    f8 = ml_dtypes.float8_e4m3

    x = np.asarray(inputs["x"], dtype=np.float32)
    WQ = np.asarray(inputs["WQ"], dtype=np.float32)
    bQ = np.asarray(inputs["bQ"], dtype=np.float32)
    WK = np.asarray(inputs["WK"], dtype=np.float32)
    bK = np.asarray(inputs["bK"], dtype=np.float32)
    WV = np.asarray(inputs["WV"], dtype=np.float32)
    bV = np.asarray(inputs["bV"], dtype=np.float32)
    WO = np.asarray(inputs["WO"], dtype=np.float32)
    bO = np.asarray(inputs["bO"], dtype=np.float32)

    from concourse.bass_utils import run_bass_kernel_spmd

    if "nc" not in _prog:
        _prog["nc"] = _build()
    nc = _prog["nc"]

    def pair_layout(Wc):
        # [1024, 512] -> [512, 1024]: out[g*128+p, par*512+m] = Wc[g*256+par*128+p, m]
        return np.ascontiguousarray(
            Wc.reshape(4, 2, 128, GD).transpose(0, 2, 1, 3).reshape(512, 1024))

    in_maps = []
    for c in range(NCORES):
        b, g = c // 2, c % 2
        sl = slice(g * GD, (g + 1) * GD)
        xb = np.ascontiguousarray(x[b])
        in_maps.append({
            "x": xb.astype(np.float16),
            "wq8": pair_layout(WQ[:, sl]).astype(f8),
            "wk8": pair_layout(WK[:, sl]).astype(f8),
            "wv8": pair_layout(WV[:, sl]).astype(f8),
            "wv16": np.ascontiguousarray(WV[:, sl]).astype(np.float16),
            "bq": np.ascontiguousarray(bQ[sl]).reshape(1, GD).astype(np.float16),
            "bk": np.ascontiguousarray(bK[sl]).reshape(1, GD).astype(np.float16),
            "wo": WO.astype(np.float16),
        })
    _prog["in_maps"] = in_maps
    res = run_bass_kernel_spmd(nc, in_maps, core_ids=list(range(NCORES)))
    parts = [r["out"] for r in res.results]

    extra = bV.reshape(H, DH).sum(0) @ WO + np.float32(H) * bO
    out = np.empty((B, S, D), dtype=np.float32)
    for b in range(B):
        out[b] = parts[2 * b] + parts[2 * b + 1] + extra
    return out
